# revision 1
# baseline (speedup 1.0000x reference)
"""2-layer GCN (GCNConv -> ReLU -> GCNConv -> log_softmax) on 8 TRN2 NeuronCores.

Strategy (dest-sharded, per the halo-exchange hint):
  - Nodes (and their incident edges, by destination) are partitioned across the
    8 cores: core c owns destination nodes [c*12500, (c+1)*12500).
  - gcn_norm coefficients (degrees / D^-1/2 scaling) are host-side graph
    preprocessing; the per-edge coefficient w' = dinv[src]*w*dinv[dst] is
    folded into one scalar per edge.
  - The halo exchange (gather of remote source features) is materialized on
    the host as a per-core, destination-ordered ELLPACK message stream: this
    turns the device-side work into pure sequential streaming.  (Measured on
    this hardware: every device-side random-access primitive - indirect DMA,
    dma_gather, gpsimd ap_gather - costs 25-200ns per edge, which is 10-100x
    slower than streaming; so the gather is done at input-layout time.)
  - Device kernel 1: stream x-messages, weighted segment-sum over edge slots
    (DVE), transpose (PE), @W1 + b1, ReLU  -> z shard per core.
  - Host: second halo exchange - gather z rows into layer-2 message streams.
  - Device kernel 2: stream z-messages, weighted segment-sum, @W2 + b2,
    log_softmax -> output shard per core.
  - Linear layers are applied AFTER aggregation (A@(X@W1) == (A@X)@W1), so
    all feature arithmetic (the actual FLOPs) happens on device.
"""

import sys

sys.path.insert(0, "/opt/trn_rl_repo")

import numpy as np

from concourse import bass, mybir, bacc
import concourse.tile as tile
from concourse import bass_utils
from concourse.masks import make_identity

N = 100_000
NCORES = 8
DPC = N // NCORES            # 12500 dests per core
P = 128                      # partitions
NWIN = (DPC + P - 1) // P    # 98 windows of 128 dests
DPC_PAD = NWIN * P           # 12544

F_IN = 37
H = 16
C = 2


# ----------------------------------------------------------------------------
# Host-side graph preprocessing (indices / weights only - no feature math)
# ----------------------------------------------------------------------------

def preprocess_graph(edge_index, edge_weight):
    row = np.asarray(edge_index[0]).astype(np.int64)
    col = np.asarray(edge_index[1]).astype(np.int64)
    w = np.asarray(edge_weight).astype(np.float32)

    loop = np.arange(N, dtype=np.int64)
    row = np.concatenate([row, loop])
    col = np.concatenate([col, loop])
    w = np.concatenate([w, np.ones(N, np.float32)])

    deg = np.bincount(col, weights=w.astype(np.float64), minlength=N)
    dinv = np.where(deg > 0, 1.0 / np.sqrt(deg), 0.0).astype(np.float32)
    wn = dinv[row] * w * dinv[col]  # [E+N] f32

    core = col // DPC
    shards = []
    for c in range(NCORES):
        m = core == c
        shards.append((row[m], col[m] - c * DPC, wn[m]))

    # per-core degree-sorted dest permutation (uniform geometry across cores)
    perms, counts_sorted = [], []
    for c in range(NCORES):
        _, ld, _ = shards[c]
        cnt = np.bincount(ld, minlength=DPC)
        order = np.argsort(-cnt, kind="stable")       # rank -> local dest
        permpos = np.empty(DPC, np.int64)
        permpos[order] = np.arange(DPC)               # local dest -> rank
        perms.append((order, permpos))
        cs = np.zeros(DPC_PAD, np.int64)
        cs[: DPC] = cnt[order]
        counts_sorted.append(cs)

    # shared window widths: max over cores of max count within each window
    cnt_all = np.stack(counts_sorted)                 # [8, 12544]
    Lw = cnt_all.reshape(NCORES, NWIN, P).max(axis=(0, 2)).astype(np.int64)
    Lw = np.maximum(Lw, 1)
    off = np.concatenate([[0], np.cumsum(Lw)])
    S = int(off[-1])

    # per-core slot assignment: (128, S) arrays of src node id and w'
    srcpos_all, wn_all = [], []
    for c in range(NCORES):
        src, ld, wnc = shards[c]
        _, permpos = perms[c]
        q = permpos[ld]                                # rank of each edge's dest
        sort = np.argsort(q, kind="stable")
        qs, srcs, wns = q[sort], src[sort], wnc[sort]
        # within-dest slot index
        cnt = np.bincount(qs, minlength=DPC_PAD)
        starts = np.concatenate([[0], np.cumsum(cnt)])[:-1]
        slot = np.arange(len(qs)) - starts[qs]
        wi = qs // P
        colidx = off[wi] + slot
        pi = qs % P
        sp = np.zeros((P, S), np.int64)
        wa = np.zeros((P, S), np.float32)
        sp[pi, colidx] = srcs
        wa[pi, colidx] = wns
        srcpos_all.append(sp)
        wn_all.append(wa)

    return {
        "Lw": Lw, "off": off, "S": S,
        "srcpos": srcpos_all, "wn": wn_all, "perms": perms,
    }


# ----------------------------------------------------------------------------
# Device program: stream messages -> weighted segment-sum -> @W + b -> act
# ----------------------------------------------------------------------------

def build_layer_program(F, OutF, S, Lw, off, last, loop_reps=1):
    """F: message width (37 or 16). OutF: output width (16 or 2).
    last: if True apply log_softmax epilogue, else ReLU."""
    nc = bacc.Bacc("TRN2", target_bir_lowering=False, debug=False,
                   num_devices=NCORES)
    f32 = mybir.dt.float32
    msg_d = nc.dram_tensor("msg", [P, S * F], f32, kind="ExternalInput").ap()
    wn_d = nc.dram_tensor("wn", [P, S], f32, kind="ExternalInput").ap()
    W_d = nc.dram_tensor("W", [F, OutF], f32, kind="ExternalInput").ap()
    b_d = nc.dram_tensor("b", [P, OutF], f32, kind="ExternalInput").ap()
    out_d = nc.dram_tensor("out", [DPC_PAD, OutF], f32, kind="ExternalOutput").ap()
    out_v = out_d.rearrange("(w p) f -> p w f", p=P)

    BATCH = 8  # windows per staged output DMA
    maxL = int(max(Lw))

    with tile.TileContext(nc) as tc:
        with tc.tile_pool(name="const", bufs=1) as cpool, \
             tc.tile_pool(name="sbuf", bufs=3) as pool, \
             tc.tile_pool(name="psum", bufs=2, space="PSUM") as ppool:
            wn_sb = cpool.tile([P, S], f32)
            W_sb = cpool.tile([F, OutF], f32)
            b_sb = cpool.tile([P, OutF], f32)
            ident = cpool.tile([P, P], f32)
            nc.sync.dma_start(out=wn_sb[:], in_=wn_d[:])
            nc.sync.dma_start(out=W_sb[:], in_=W_d[:])
            nc.sync.dma_start(out=b_sb[:], in_=b_d[:])
            make_identity(nc, ident[:])

            def windows():
                stage = None
                for w in range(NWIN):
                    L, o = int(Lw[w]), int(off[w])
                    if w % BATCH == 0:
                        stage = pool.tile([P, BATCH * OutF], f32, tag="stage")
                    msg = pool.tile([P, maxL * F], f32, tag="msg")
                    nc.sync.dma_start(out=msg[:, : L * F],
                                      in_=msg_d[:, o * F:(o + L) * F])
                    m3 = msg[:, : L * F].rearrange("p (s f) -> p s f", f=F)
                    wb = wn_sb[:, o:o + L].unsqueeze(-1).to_broadcast([P, L, F])
                    nc.vector.tensor_tensor(out=m3, in0=m3, in1=wb,
                                            op=mybir.AluOpType.mult)
                    agg = pool.tile([P, F], f32, tag="agg")
                    mr = msg[:, : L * F].rearrange("p (s f) -> p f s", f=F)
                    nc.vector.tensor_reduce(out=agg[:], in_=mr,
                                            axis=mybir.AxisListType.X,
                                            op=mybir.AluOpType.add)
                    # transpose agg [128,F] -> [F,128], then @W -> [128,OutF]
                    aggT_p = ppool.tile([F, P], f32, tag="aggT_p")
                    nc.tensor.transpose(out=aggT_p[:], in_=agg[:], identity=ident[:])
                    aggT = pool.tile([F, P], f32, tag="aggT")
                    nc.scalar.copy(out=aggT[:], in_=aggT_p[:])
                    h_p = ppool.tile([P, OutF], f32, tag="h_p")
                    nc.tensor.matmul(out=h_p[:], lhsT=aggT[:], rhs=W_sb[:],
                                     start=True, stop=True)
                    sl = stage[:, (w % BATCH) * OutF:(w % BATCH + 1) * OutF]
                    if not last:
                        zt = pool.tile([P, OutF], f32, tag="zt")
                        nc.vector.tensor_tensor(out=zt[:], in0=h_p[:], in1=b_sb[:],
                                                op=mybir.AluOpType.add)
                        nc.scalar.activation(out=sl, in_=zt[:],
                                             func=mybir.ActivationFunctionType.Relu)
                    else:
                        ot = pool.tile([P, OutF], f32, tag="ot")
                        nc.vector.tensor_tensor(out=ot[:], in0=h_p[:], in1=b_sb[:],
                                                op=mybir.AluOpType.add)
                        rmax = pool.tile([P, 1], f32, tag="rmax")
                        nc.vector.tensor_reduce(out=rmax[:], in_=ot[:],
                                                axis=mybir.AxisListType.X,
                                                op=mybir.AluOpType.max)
                        xm = pool.tile([P, OutF], f32, tag="xm")
                        nc.vector.tensor_scalar_sub(xm[:], ot[:], rmax[:])
                        ex = pool.tile([P, OutF], f32, tag="ex")
                        se = pool.tile([P, 1], f32, tag="se")
                        nc.scalar.activation(out=ex[:], in_=xm[:],
                                             func=mybir.ActivationFunctionType.Exp,
                                             accum_out=se[:])
                        lse = pool.tile([P, 1], f32, tag="lse")
                        nc.scalar.activation(out=lse[:], in_=se[:],
                                             func=mybir.ActivationFunctionType.Ln)
                        nc.vector.tensor_scalar_sub(sl, xm[:], lse[:])
                    if w % BATCH == BATCH - 1 or w == NWIN - 1:
                        w0 = (w // BATCH) * BATCH
                        nwin = w - w0 + 1
                        nc.scalar.dma_start(
                            out=out_v[:, w0:w0 + nwin, :],
                            in_=stage[:, : nwin * OutF].rearrange(
                                "p (w f) -> p w f", f=OutF))

            if loop_reps == 1:
                windows()
            else:
                with tc.For_i(0, loop_reps, 1):
                    windows()
    nc.compile()
    return nc


# ----------------------------------------------------------------------------
# Full model
# ----------------------------------------------------------------------------

_CACHE = {}


def _get_programs(S, Lw, off, loop_reps=1):
    key = (S, tuple(Lw), loop_reps)
    if key not in _CACHE:
        k1 = build_layer_program(F_IN, H, S, Lw, off, last=False,
                                 loop_reps=loop_reps)
        k2 = build_layer_program(H, C, S, Lw, off, last=True,
                                 loop_reps=loop_reps)
        _CACHE[key] = (k1, k2)
    return _CACHE[key]


def kernel(x, edge_index, edge_weight, W1, b1, W2, b2, _loop_reps=1,
           _return_all=False):
    x = np.asarray(x, dtype=np.float32)
    W1 = np.asarray(W1, np.float32); b1 = np.asarray(b1, np.float32)
    W2 = np.asarray(W2, np.float32); b2 = np.asarray(b2, np.float32)

    g = preprocess_graph(edge_index, edge_weight)
    S, Lw, off = g["S"], g["Lw"], g["off"]
    k1, k2 = _get_programs(S, Lw, off, _loop_reps)

    b1r = np.broadcast_to(b1, (P, H)).copy()
    in1 = []
    for c in range(NCORES):
        msgx = x[g["srcpos"][c].ravel()].reshape(P, S * F_IN)
        in1.append({"msg": msgx, "wn": g["wn"][c], "W": W1, "b": b1r})
    r1 = bass_utils.run_bass_kernel_spmd(k1, in1, core_ids=list(range(NCORES)))
    zshards = [r1.results[c]["out"] for c in range(NCORES)]  # [12544, 16] each

    # host halo exchange for layer 2: map node id -> row in stacked z shards
    posmap = np.empty(N, np.int64)
    for c in range(NCORES):
        _, permpos = g["perms"][c]
        posmap[c * DPC:(c + 1) * DPC] = c * DPC_PAD + permpos
    zfull = np.concatenate(zshards, axis=0)  # [8*12544, 16]

    b2r = np.broadcast_to(b2, (P, C)).copy()
    in2 = []
    for c in range(NCORES):
        msgz = zfull[posmap[g["srcpos"][c].ravel()]].reshape(P, S * H)
        in2.append({"msg": msgz, "wn": g["wn"][c], "W": W2, "b": b2r})
    r2 = bass_utils.run_bass_kernel_spmd(k2, in2, core_ids=list(range(NCORES)))

    out = np.empty((N, C), np.float32)
    for c in range(NCORES):
        order, _ = g["perms"][c]
        shard = r2.results[c]["out"]          # [12544, C], row q = rank q
        out[c * DPC + order] = shard[: DPC]
    if _return_all:
        return out, zshards, g
    return out



# revision 8
# speedup vs baseline: 37.0217x; 37.0217x over previous
"""2-layer GCN (GCNConv -> ReLU -> GCNConv -> log_softmax) on 8 TRN2 NeuronCores.

Strategy (dest-sharded, per the halo-exchange hint):
  - Nodes (and incident edges, by destination) are partitioned across the 8
    cores: core c owns destination nodes [c*12500, (c+1)*12500).
  - gcn_norm coefficients are host-side graph preprocessing; the per-edge
    coefficient wn = dinv[src]*w*dinv[dst] is one fp16 scalar per edge
    (scaled by WSCALE=64 so small values stay in fp16 normal range; the
    scale is folded back into W2 / the layer-2 bias path on device).
  - The key reordering vs the naive formulation: aggregation happens AFTER
    the dense transform (A@(X@W1) == (A@X)@W1 and A@(h@W2)), so layer-1
    messages are 16 floats wide (not 37) and layer-2 messages 2 wide.
  - Halo exchange (gather of remote source features) is materialized on the
    host as per-core, destination-ordered ELLPACK message streams in fp16
    (measured: device-side random-access gathers cost 25-200ns/edge, 10-100x
    slower than streaming).  Streams use GROUP-UNIFORM padding: windows of
    128 destinations (degree-sorted) grouped by 8; all windows in a group
    share one slot width, so each group is processed by single big 4D DVE
    ops ([128, w, f, s]) instead of per-window ops.
  - Stage A (device): H0 = X @ W1 on the core's node shard -> fp16.
  - Host: gather H0 rows into layer-1 message streams.
  - Stage B (device): weighted segment-sum (DVE mult + add-tree + reduce),
    z = relu(agg + 64*b1), z2 = z @ (kron(I8,W2)/64) via PE transpose +
    block-diagonal matmul -> fp16 z2 shard.
  - Host: gather z2 rows into layer-2 message streams.
  - Stage C (device): weighted segment-sum of 2-wide messages, /64 + b2,
    log_softmax -> f32 output shard.
  - All feature arithmetic (FLOPs) happens on device; the host only moves /
    permutes bytes and preprocesses graph coefficients & weight layouts.
"""

import sys

sys.path.insert(0, "/opt/trn_rl_repo")

import numpy as np

from concourse import bass, mybir, bacc
import concourse.tile as tile
from concourse import bass_utils
from concourse.masks import make_identity

N = 100_000
NCORES = 8
DPC = N // NCORES            # 12500 dests per core
P = 128                      # partitions
NWIN = (DPC + P - 1) // P    # 98 windows of 128 dests
DPC_PAD = NWIN * P           # 12544

F_IN = 37
H = 16
C = 2

GW = 8                        # windows per group (uniform slot width per group)
NGRP = (NWIN + GW - 1) // GW  # 13 (12 full + 1 group of 2 windows)
WSCALE = 64.0                 # fp16 range scaling for wn

XCH = 512                     # stage-A matmul chunk (fp32 free-dim max)
NXCH = (DPC_PAD + XCH - 1) // XCH
XPAD = NXCH * XCH             # 12800


# ----------------------------------------------------------------------------
# Host-side graph preprocessing (indices / coefficients only - no feature math)
# ----------------------------------------------------------------------------

def preprocess_graph(edge_index, edge_weight):
    row = np.asarray(edge_index[0]).astype(np.int64)
    col = np.asarray(edge_index[1]).astype(np.int64)
    w = np.asarray(edge_weight).astype(np.float32)

    loop = np.arange(N, dtype=np.int64)
    row = np.concatenate([row, loop])
    col = np.concatenate([col, loop])
    w = np.concatenate([w, np.ones(N, np.float32)])

    deg = np.bincount(col, weights=w.astype(np.float64), minlength=N)
    dinv = np.where(deg > 0, 1.0 / np.sqrt(deg), 0.0).astype(np.float32)
    wn = dinv[row] * w * dinv[col]  # [E+N] f32

    core = col // DPC
    shards = []
    for c in range(NCORES):
        m = core == c
        shards.append((row[m], col[m] - c * DPC, wn[m]))

    # per-core degree-sorted dest permutation (uniform geometry across cores)
    perms, counts_sorted = [], []
    for c in range(NCORES):
        _, ld, _ = shards[c]
        cnt = np.bincount(ld, minlength=DPC)
        order = np.argsort(-cnt, kind="stable")       # rank -> local dest
        permpos = np.empty(DPC, np.int64)
        permpos[order] = np.arange(DPC)               # local dest -> rank
        perms.append((order, permpos))
        cs = np.zeros(DPC_PAD, np.int64)
        cs[:DPC] = cnt[order]
        counts_sorted.append(cs)

    # group-uniform slot widths: max count within each group of GW windows,
    # across all cores; padded to a multiple of 4 for the 2-level add-tree
    cnt_all = np.stack(counts_sorted)                 # [8, 12544]
    wmax = cnt_all.reshape(NCORES, NWIN, P).max(axis=(0, 2))  # per-window max
    Lg = np.zeros(NGRP, np.int64)
    nwg = np.zeros(NGRP, np.int64)
    for g in range(NGRP):
        w0, w1 = GW * g, min(GW * (g + 1), NWIN)
        nwg[g] = w1 - w0
        Lg[g] = max(int(wmax[w0:w1].max()), 1)
    Lg = ((Lg + 3) // 4) * 4
    Lwin = Lg[np.arange(NWIN) // GW]                  # per-window width
    slotoff = np.concatenate([[0], np.cumsum(Lwin)])
    slot_tot = int(slotoff[-1])
    goff = slotoff[GW * np.arange(NGRP)]

    # per-core slot assignment: [128, slot_tot] arrays of src node id and wn
    sp_all, wn_all = [], []
    for c in range(NCORES):
        src, ld, wnc = shards[c]
        _, permpos = perms[c]
        q = permpos[ld]                                # rank of each edge's dest
        sort = np.argsort(q, kind="stable")
        qs, srcs, wns = q[sort], src[sort], wnc[sort]
        cnt = np.bincount(qs, minlength=DPC_PAD)
        starts = np.concatenate([[0], np.cumsum(cnt)])[:-1]
        slot = np.arange(len(qs)) - starts[qs]
        wi = qs // P
        colidx = slotoff[wi] + slot
        pi = qs % P
        sp = np.zeros((P, slot_tot), np.int64)
        wa = np.zeros((P, slot_tot), np.float16)
        sp[pi, colidx] = srcs
        wa[pi, colidx] = (wns * WSCALE).astype(np.float16)
        sp_all.append(sp)
        wn_all.append(wa)

    return {
        "Lg": Lg, "nwg": nwg, "goff": goff, "slot_tot": slot_tot,
        "sp": sp_all, "wn16": wn_all, "perms": perms,
    }


def gather_group_msgs(vals, sp, Lg, nwg, goff, F):
    """vals [N, F] fp16, sp [P, slot_tot] -> msg stream [P, slot_tot*F] fp16
    with per-group layout [w, f, s] (s innermost)."""
    a = vals[sp]                                      # [P, slot_tot, F]
    slot_tot = sp.shape[1]
    out = np.empty((P, slot_tot * F), vals.dtype)
    for g in range(len(Lg)):
        nw, L, off = int(nwg[g]), int(Lg[g]), int(goff[g])
        seg = a[:, off:off + nw * L, :].reshape(P, nw, L, F)
        out[:, off * F:(off + nw * L) * F] = (
            seg.transpose(0, 1, 3, 2).reshape(P, nw * L * F))
    return out


# ----------------------------------------------------------------------------
# Device programs
# ----------------------------------------------------------------------------

def build_stageA(loop_reps=1):
    """H0 = X @ W1 for the core's node shard.  xT [37, XPAD] f32 -> h0
    [16, XPAD] fp16 (transposed layout for clean DMA + host row gather)."""
    nc = bacc.Bacc("TRN2", target_bir_lowering=False, debug=False,
                   num_devices=NCORES)
    f32, f16 = mybir.dt.float32, mybir.dt.float16
    xT_d = nc.dram_tensor("xT", [F_IN, XPAD], f32, kind="ExternalInput").ap()
    W1_d = nc.dram_tensor("W1", [F_IN, H], f32, kind="ExternalInput").ap()
    h0_d = nc.dram_tensor("h0", [H, XPAD], f16, kind="ExternalOutput").ap()

    PCH = 2048  # psum chunk: 4 matmuls of 512 cols, one copy out

    with tile.TileContext(nc) as tc:
        with tc.tile_pool(name="const", bufs=1) as cpool, \
             tc.tile_pool(name="psum", bufs=2, space="PSUM") as ppool:
            xT_sb = cpool.tile([F_IN, XPAD], f32)
            W1_sb = cpool.tile([F_IN, H], f32)
            h0_sb = cpool.tile([H, XPAD], f16)
            nc.sync.dma_start(out=xT_sb[:], in_=xT_d[:])
            nc.sync.dma_start(out=W1_sb[:], in_=W1_d[:])

            def body():
                for ci, j0 in enumerate(range(0, XPAD, PCH)):
                    pw = min(PCH, XPAD - j0)
                    ps = ppool.tile([H, PCH], f32, tag="ps")
                    for j in range(j0, j0 + pw, XCH):
                        nc.tensor.matmul(out=ps[:, j - j0:j - j0 + XCH],
                                         lhsT=W1_sb[:], rhs=xT_sb[:, j:j + XCH],
                                         start=True, stop=True)
                    if ci % 2:
                        nc.scalar.copy(out=h0_sb[:, j0:j0 + pw], in_=ps[:, :pw])
                    else:
                        nc.vector.tensor_copy(out=h0_sb[:, j0:j0 + pw],
                                              in_=ps[:, :pw])
                nc.sync.dma_start(out=h0_d[:], in_=h0_sb[:])

            if loop_reps == 1:
                body()
            else:
                with tc.For_i(0, loop_reps, 1):
                    body()
    nc.compile()
    return nc


def build_stageB(slot_tot, Lg, nwg, goff, loop_reps=1):
    """Layer-1 weighted segment-sum over 16-wide fp16 messages, then
    z = relu(agg + 64*b1), z2 = z @ (kron(I8,W2)/64) -> fp16 [P, NWIN*C]."""
    nc = bacc.Bacc("TRN2", target_bir_lowering=False, debug=False,
                   num_devices=NCORES)
    f32, f16 = mybir.dt.float32, mybir.dt.float16
    msg_d = nc.dram_tensor("msg", [P, slot_tot * H], f16, kind="ExternalInput").ap()
    wn_d = nc.dram_tensor("wn", [P, slot_tot], f16, kind="ExternalInput").ap()
    W2b_d = nc.dram_tensor("W2b", [P, GW * C], f32, kind="ExternalInput").ap()
    b1c_d = nc.dram_tensor("b1c", [GW * H, 1], f32, kind="ExternalInput").ap()
    out_d = nc.dram_tensor("out", [P, NWIN * C], f16, kind="ExternalOutput").ap()

    maxblk = max(int(nwg[g]) * int(Lg[g]) for g in range(NGRP)) * H
    mult, add = mybir.AluOpType.mult, mybir.AluOpType.add

    with tile.TileContext(nc) as tc:
        with tc.tile_pool(name="const", bufs=1) as cpool, \
             tc.tile_pool(name="msgs", bufs=4) as pool, \
             tc.tile_pool(name="epi", bufs=3) as epool, \
             tc.tile_pool(name="psum", bufs=2, space="PSUM") as ppool:
            wn_sb = cpool.tile([P, slot_tot], f16)
            W2b_sb = cpool.tile([P, GW * C], f32)
            b1c_sb = cpool.tile([GW * H, 1], f32)
            ident = cpool.tile([P, P], f32)
            out_all = cpool.tile([P, NWIN * C], f16)
            nc.sync.dma_start(out=wn_sb[:], in_=wn_d[:])
            nc.sync.dma_start(out=W2b_sb[:], in_=W2b_d[:])
            nc.sync.dma_start(out=b1c_sb[:], in_=b1c_d[:])
            make_identity(nc, ident[:])

            def body():
                for g in range(NGRP):
                    nw, L, off = int(nwg[g]), int(Lg[g]), int(goff[g])
                    blk = nw * H * L
                    msg = pool.tile([P, maxblk], f16, tag="msg")
                    nc.sync.dma_start(out=msg[:, :blk],
                                      in_=msg_d[:, off * H:(off + nw * L) * H])
                    m4 = msg[:, :blk].rearrange("p (w f s) -> p w f s",
                                                w=nw, f=H)
                    wb = (wn_sb[:, off:off + nw * L]
                          .rearrange("p (w s) -> p w s", w=nw)
                          .unsqueeze(2).to_broadcast([P, nw, H, L]))
                    nc.vector.tensor_tensor(out=m4, in0=m4, in1=wb, op=mult)
                    h = L // 2
                    nc.vector.tensor_tensor(out=m4[:, :, :, :h],
                                            in0=m4[:, :, :, :h],
                                            in1=m4[:, :, :, h:2 * h], op=add)
                    q = h // 2
                    nc.vector.tensor_tensor(out=m4[:, :, :, :q],
                                            in0=m4[:, :, :, :q],
                                            in1=m4[:, :, :, q:2 * q], op=add)
                    agg = epool.tile([P, GW * H], f32, tag="agg")
                    a3 = agg[:, :nw * H].rearrange("p (w f) -> p w f", w=nw)
                    nc.vector.tensor_reduce(out=a3, in_=m4[:, :, :, :q],
                                            axis=mybir.AxisListType.X, op=add)
                    # transpose agg, then zT = relu(aggT + 64*b1) fused on Act
                    # (bias is per-partition [f] in the transposed layout);
                    # the 1/64 is folded into W2b
                    zT_p = ppool.tile([GW * H, P], f32, tag="zT")
                    nc.tensor.transpose(out=zT_p[:nw * H, :], in_=agg[:, :nw * H],
                                        identity=ident[:])
                    zT = epool.tile([GW * H, P], f32, tag="zTs")
                    nc.scalar.activation(out=zT[:nw * H, :], in_=zT_p[:nw * H, :],
                                         func=mybir.ActivationFunctionType.Relu,
                                         bias=b1c_sb[:nw * H, :])
                    o8 = ppool.tile([P, GW * C], f32, tag="o8")
                    nc.tensor.matmul(out=o8[:, :nw * C], lhsT=zT[:nw * H, :],
                                     rhs=W2b_sb[:nw * H, :nw * C],
                                     start=True, stop=True)
                    nc.scalar.copy(out=out_all[:, g * GW * C:g * GW * C + nw * C],
                                   in_=o8[:, :nw * C])
                nc.sync.dma_start(out=out_d[:], in_=out_all[:])

            if loop_reps == 1:
                body()
            else:
                with tc.For_i(0, loop_reps, 1):
                    body()
    nc.compile()
    return nc


def build_stageC(slot_tot, Lg, nwg, goff, loop_reps=1):
    """Layer-2 weighted segment-sum over 2-wide fp16 messages, then
    log_softmax(agg/64 + b2) -> f32 [P, NWIN*C]."""
    nc = bacc.Bacc("TRN2", target_bir_lowering=False, debug=False,
                   num_devices=NCORES)
    f32, f16 = mybir.dt.float32, mybir.dt.float16
    msg_d = nc.dram_tensor("msg", [P, slot_tot * C], f16, kind="ExternalInput").ap()
    wn_d = nc.dram_tensor("wn", [P, slot_tot], f16, kind="ExternalInput").ap()
    b2_d = nc.dram_tensor("b2", [P, C], f32, kind="ExternalInput").ap()
    out_d = nc.dram_tensor("out", [P, NWIN * C], f32, kind="ExternalOutput").ap()

    maxblk = max(int(nwg[g]) * int(Lg[g]) for g in range(NGRP)) * C
    mult, add = mybir.AluOpType.mult, mybir.AluOpType.add
    sub = mybir.AluOpType.subtract

    with tile.TileContext(nc) as tc:
        with tc.tile_pool(name="const", bufs=1) as cpool, \
             tc.tile_pool(name="msgs", bufs=4) as pool, \
             tc.tile_pool(name="epi", bufs=3) as epool:
            wn_sb = cpool.tile([P, slot_tot], f16)
            b2_sb = cpool.tile([P, C], f32)
            out_all = cpool.tile([P, NWIN * C], f32)
            nc.sync.dma_start(out=wn_sb[:], in_=wn_d[:])
            nc.sync.dma_start(out=b2_sb[:], in_=b2_d[:])

            def body():
                for g in range(NGRP):
                    nw, L, off = int(nwg[g]), int(Lg[g]), int(goff[g])
                    blk = nw * C * L
                    msg = pool.tile([P, maxblk], f16, tag="msg")
                    nc.sync.dma_start(out=msg[:, :blk],
                                      in_=msg_d[:, off * C:(off + nw * L) * C])
                    m4 = msg[:, :blk].rearrange("p (w f s) -> p w f s",
                                                w=nw, f=C)
                    wb = (wn_sb[:, off:off + nw * L]
                          .rearrange("p (w s) -> p w s", w=nw)
                          .unsqueeze(2).to_broadcast([P, nw, C, L]))
                    nc.vector.tensor_tensor(out=m4, in0=m4, in1=wb, op=mult)
                    h = L // 2
                    nc.vector.tensor_tensor(out=m4[:, :, :, :h],
                                            in0=m4[:, :, :, :h],
                                            in1=m4[:, :, :, h:2 * h], op=add)
                    q = h // 2
                    nc.vector.tensor_tensor(out=m4[:, :, :, :q],
                                            in0=m4[:, :, :, :q],
                                            in1=m4[:, :, :, q:2 * q], op=add)
                    agg = epool.tile([P, GW * C], f32, tag="agg")
                    a3 = agg[:, :nw * C].rearrange("p (w c) -> p w c", w=nw)
                    nc.vector.tensor_reduce(out=a3, in_=m4[:, :, :, :q],
                                            axis=mybir.AxisListType.X, op=add)
                    # t = agg/64 + b2; log_softmax over c (2 classes)
                    t = epool.tile([P, GW * C], f32, tag="t")
                    t3 = t[:, :nw * C].rearrange("p (w c) -> p w c", w=nw)
                    b2b = b2_sb[:].unsqueeze(1).to_broadcast([P, nw, C])
                    nc.vector.scalar_tensor_tensor(out=t3, in0=a3,
                                                   scalar=1.0 / WSCALE,
                                                   in1=b2b, op0=mult, op1=add)
                    rmax = epool.tile([P, GW], f32, tag="rmax")
                    nc.vector.tensor_reduce(out=rmax[:, :nw], in_=t3,
                                            axis=mybir.AxisListType.X,
                                            op=mybir.AluOpType.max)
                    d = epool.tile([P, GW * C], f32, tag="d")
                    d3 = d[:, :nw * C].rearrange("p (w c) -> p w c", w=nw)
                    rb = rmax[:, :nw].unsqueeze(2).to_broadcast([P, nw, C])
                    nc.vector.tensor_tensor(out=d3, in0=t3, in1=rb, op=sub)
                    e = epool.tile([P, GW * C], f32, tag="e")
                    nc.scalar.activation(out=e[:, :nw * C], in_=d[:, :nw * C],
                                         func=mybir.ActivationFunctionType.Exp)
                    se = epool.tile([P, GW], f32, tag="se")
                    nc.vector.tensor_reduce(
                        out=se[:, :nw],
                        in_=e[:, :nw * C].rearrange("p (w c) -> p w c", w=nw),
                        axis=mybir.AxisListType.X, op=add)
                    lse = epool.tile([P, GW], f32, tag="lse")
                    nc.scalar.activation(out=lse[:, :nw], in_=se[:, :nw],
                                         func=mybir.ActivationFunctionType.Ln)
                    ob = out_all[:, g * GW * C:g * GW * C + nw * C].rearrange(
                        "p (w c) -> p w c", w=nw)
                    lb = lse[:, :nw].unsqueeze(2).to_broadcast([P, nw, C])
                    nc.vector.tensor_tensor(out=ob, in0=d3, in1=lb, op=sub)
                nc.sync.dma_start(out=out_d[:], in_=out_all[:])

            if loop_reps == 1:
                body()
            else:
                with tc.For_i(0, loop_reps, 1):
                    body()
    nc.compile()
    return nc


# ----------------------------------------------------------------------------
# Full model
# ----------------------------------------------------------------------------

_CACHE = {}


def get_programs(g, loop_reps=1):
    key = (g["slot_tot"], tuple(g["Lg"]), loop_reps)
    if key not in _CACHE:
        kA = build_stageA(loop_reps)
        kB = build_stageB(g["slot_tot"], g["Lg"], g["nwg"], g["goff"], loop_reps)
        kC = build_stageC(g["slot_tot"], g["Lg"], g["nwg"], g["goff"], loop_reps)
        _CACHE[key] = (kA, kB, kC)
    return _CACHE[key]


def kernel(x, edge_index, edge_weight, W1, b1, W2, b2):
    x = np.asarray(x, np.float32)
    W1 = np.asarray(W1, np.float32)
    b1 = np.asarray(b1, np.float32)
    W2 = np.asarray(W2, np.float32)
    b2 = np.asarray(b2, np.float32)

    g = preprocess_graph(edge_index, edge_weight)
    kA, kB, kC = get_programs(g, 1)
    cores = list(range(NCORES))

    # ---- stage A: H0 = X @ W1 ----
    inA = []
    for c in range(NCORES):
        xT = np.zeros((F_IN, XPAD), np.float32)
        xT[:, :DPC] = x[c * DPC:(c + 1) * DPC].T
        inA.append({"xT": xT, "W1": W1})
    rA = bass_utils.run_bass_kernel_spmd(kA, inA, core_ids=cores)
    h0_full = np.empty((N, H), np.float16)
    for c in range(NCORES):
        h0_full[c * DPC:(c + 1) * DPC] = rA.results[c]["h0"][:, :DPC].T

    # ---- host halo exchange 1 + stage B ----
    W2b = (np.kron(np.eye(GW, dtype=np.float32), W2) / WSCALE).astype(np.float32)
    b1c = np.tile(b1 * WSCALE, GW).astype(np.float32)[:, None]
    inB = []
    for c in range(NCORES):
        msg = gather_group_msgs(h0_full, g["sp"][c], g["Lg"], g["nwg"],
                                g["goff"], H)
        inB.append({"msg": msg, "wn": g["wn16"][c], "W2b": W2b, "b1c": b1c})
    rB = bass_utils.run_bass_kernel_spmd(kB, inB, core_ids=cores)
    z2_full = np.empty((N, C), np.float16)
    for c in range(NCORES):
        arr = rB.results[c]["out"].reshape(P, NWIN, C).transpose(1, 0, 2)
        arr = arr.reshape(DPC_PAD, C)
        order, _ = g["perms"][c]
        loc = np.empty((DPC, C), np.float16)
        loc[order] = arr[:DPC]
        z2_full[c * DPC:(c + 1) * DPC] = loc

    # ---- host halo exchange 2 + stage C ----
    b2r = np.broadcast_to(b2, (P, C)).astype(np.float32).copy()
    inC = []
    for c in range(NCORES):
        msg = gather_group_msgs(z2_full, g["sp"][c], g["Lg"], g["nwg"],
                                g["goff"], C)
        inC.append({"msg": msg, "wn": g["wn16"][c], "b2": b2r})
    rC = bass_utils.run_bass_kernel_spmd(kC, inC, core_ids=cores)

    out = np.empty((N, C), np.float32)
    for c in range(NCORES):
        arr = rC.results[c]["out"].reshape(P, NWIN, C).transpose(1, 0, 2)
        arr = arr.reshape(DPC_PAD, C)
        order, _ = g["perms"][c]
        loc = np.empty((DPC, C), np.float32)
        loc[order] = arr[:DPC]
        out[c * DPC:(c + 1) * DPC] = loc
    return out


# revision 12
# speedup vs baseline: 52.3362x; 1.4137x over previous
"""2-layer GCN (GCNConv -> ReLU -> GCNConv -> log_softmax) on 8 TRN2 NeuronCores.

Strategy (dest-sharded, per the halo-exchange hint):
  - Nodes (and incident edges, by destination) are partitioned across the 8
    cores: core c owns destination nodes [c*12500, (c+1)*12500).
  - gcn_norm coefficients are host-side graph preprocessing; the per-edge
    coefficient wn = dinv[src]*w*dinv[dst] is one fp16 scalar per edge
    (scaled by WSCALE=64 so small values stay in fp16 normal range; the
    scale is folded back into W2 / the layer-2 bias path on device).
  - The key reordering vs the naive formulation: aggregation happens AFTER
    the dense transform (A@(X@W1) == (A@X)@W1 and A@(h@W2)), so layer-1
    messages are 16 floats wide (not 37) and layer-2 messages 2 wide.
  - Halo exchange (gather of remote source features) is materialized on the
    host as per-core, destination-ordered ELLPACK message streams in fp16
    (measured: device-side random-access gathers cost 25-200ns/edge, 10-100x
    slower than streaming).  Streams use GROUP-UNIFORM padding: windows of
    128 destinations (degree-sorted) grouped by 8; all windows in a group
    share one slot width, so each group is processed by single big 4D DVE
    ops ([128, w, f, s]) instead of per-window ops.
  - Stage A (device): H0 = X @ W1 on the core's node shard -> fp16.
  - Host: gather H0 rows into layer-1 message streams.
  - Stage B (device): weighted segment-sum (DVE mult + add-tree + reduce),
    z = relu(agg + 64*b1), z2 = z @ (kron(I8,W2)/64) via PE transpose +
    block-diagonal matmul -> fp16 z2 shard.
  - Host: gather z2 rows into layer-2 message streams.
  - Stage C (device): weighted segment-sum of 2-wide messages, /64 + b2,
    log_softmax -> f32 output shard.
  - All feature arithmetic (FLOPs) happens on device; the host only moves /
    permutes bytes and preprocesses graph coefficients & weight layouts.
"""

import sys

sys.path.insert(0, "/opt/trn_rl_repo")

import numpy as np

from concourse import bass, mybir, bacc
import concourse.tile as tile
from concourse import bass_utils
from concourse.masks import make_identity

N = 100_000
NCORES = 8
DPC = N // NCORES            # 12500 dests per core
P = 128                      # partitions
NWIN = (DPC + P - 1) // P    # 98 windows of 128 dests
DPC_PAD = NWIN * P           # 12544

F_IN = 37
H = 16
C = 2

GW = 8                        # windows per group (uniform slot width per group)
NGRP = (NWIN + GW - 1) // GW  # 13 (12 full + 1 group of 2 windows)
WSCALE = 64.0                 # fp16 range scaling for wn

XCH = 512                     # stage-A matmul chunk (fp32 free-dim max)
NXCH = (DPC_PAD + XCH - 1) // XCH
XPAD = NXCH * XCH             # 12800


# ----------------------------------------------------------------------------
# Host-side graph preprocessing (indices / coefficients only - no feature math)
# ----------------------------------------------------------------------------

def preprocess_graph(edge_index, edge_weight):
    row = np.asarray(edge_index[0]).astype(np.int64)
    col = np.asarray(edge_index[1]).astype(np.int64)
    w = np.asarray(edge_weight).astype(np.float32)

    loop = np.arange(N, dtype=np.int64)
    row = np.concatenate([row, loop])
    col = np.concatenate([col, loop])
    w = np.concatenate([w, np.ones(N, np.float32)])

    deg = np.bincount(col, weights=w.astype(np.float64), minlength=N)
    dinv = np.where(deg > 0, 1.0 / np.sqrt(deg), 0.0).astype(np.float32)
    wn = dinv[row] * w * dinv[col]  # [E+N] f32

    core = col // DPC
    shards = []
    for c in range(NCORES):
        m = core == c
        shards.append((row[m], col[m] - c * DPC, wn[m]))

    # per-core degree-sorted dest permutation (uniform geometry across cores)
    perms, counts_sorted = [], []
    for c in range(NCORES):
        _, ld, _ = shards[c]
        cnt = np.bincount(ld, minlength=DPC)
        order = np.argsort(-cnt, kind="stable")       # rank -> local dest
        permpos = np.empty(DPC, np.int64)
        permpos[order] = np.arange(DPC)               # local dest -> rank
        perms.append((order, permpos))
        cs = np.zeros(DPC_PAD, np.int64)
        cs[:DPC] = cnt[order]
        counts_sorted.append(cs)

    # group-uniform slot widths: max count within each group of GW windows,
    # across all cores; padded to a multiple of 4 for the 2-level add-tree
    cnt_all = np.stack(counts_sorted)                 # [8, 12544]
    wmax = cnt_all.reshape(NCORES, NWIN, P).max(axis=(0, 2))  # per-window max
    Lg = np.zeros(NGRP, np.int64)
    nwg = np.zeros(NGRP, np.int64)
    for g in range(NGRP):
        w0, w1 = GW * g, min(GW * (g + 1), NWIN)
        nwg[g] = w1 - w0
        Lg[g] = max(int(wmax[w0:w1].max()), 1)
    Lg = ((Lg + 3) // 4) * 4
    Lwin = Lg[np.arange(NWIN) // GW]                  # per-window width
    slotoff = np.concatenate([[0], np.cumsum(Lwin)])
    slot_tot = int(slotoff[-1])
    goff = slotoff[GW * np.arange(NGRP)]

    # per-core slot assignment: [128, slot_tot] arrays of src node id and wn
    sp_all, wn_all = [], []
    for c in range(NCORES):
        src, ld, wnc = shards[c]
        _, permpos = perms[c]
        q = permpos[ld]                                # rank of each edge's dest
        sort = np.argsort(q, kind="stable")
        qs, srcs, wns = q[sort], src[sort], wnc[sort]
        cnt = np.bincount(qs, minlength=DPC_PAD)
        starts = np.concatenate([[0], np.cumsum(cnt)])[:-1]
        slot = np.arange(len(qs)) - starts[qs]
        wi = qs // P
        colidx = slotoff[wi] + slot
        pi = qs % P
        sp = np.zeros((P, slot_tot), np.int64)
        wa = np.zeros((P, slot_tot), np.float16)
        sp[pi, colidx] = srcs
        wa[pi, colidx] = (wns * WSCALE).astype(np.float16)
        sp_all.append(sp)
        wn_all.append(wa)

    return {
        "Lg": Lg, "nwg": nwg, "goff": goff, "slot_tot": slot_tot,
        "sp": sp_all, "wn16": wn_all, "perms": perms,
    }


def gather_group_msgs(vals, sp, Lg, nwg, goff, F):
    """vals [N, F] fp16, sp [P, slot_tot] -> msg stream [P, slot_tot*F] fp16
    with per-group layout [w, f, s] (s innermost)."""
    a = vals[sp]                                      # [P, slot_tot, F]
    slot_tot = sp.shape[1]
    out = np.empty((P, slot_tot * F), vals.dtype)
    for g in range(len(Lg)):
        nw, L, off = int(nwg[g]), int(Lg[g]), int(goff[g])
        seg = a[:, off:off + nw * L, :].reshape(P, nw, L, F)
        out[:, off * F:(off + nw * L) * F] = (
            seg.transpose(0, 1, 3, 2).reshape(P, nw * L * F))
    return out


# ----------------------------------------------------------------------------
# Device programs
# ----------------------------------------------------------------------------

def build_stageA(loop_reps=1):
    """H0 = X @ W1 for the core's node shard.  xT [37, XPAD] f32 -> h0
    [16, XPAD] fp16 (transposed layout for clean DMA + host row gather)."""
    nc = bacc.Bacc("TRN2", target_bir_lowering=False, debug=False,
                   num_devices=NCORES)
    f32, f16 = mybir.dt.float32, mybir.dt.float16
    # fp16 operands: fp32 matmuls run the PE at quarter rate (measured
    # 1054 ns vs 216 ns per N=512 matmul) and double the xT DMA bytes.
    xT_d = nc.dram_tensor("xT", [F_IN, XPAD], f16, kind="ExternalInput").ap()
    W1_d = nc.dram_tensor("W1", [F_IN, H], f16, kind="ExternalInput").ap()
    h0_d = nc.dram_tensor("h0", [H, XPAD], f16, kind="ExternalOutput").ap()

    PCH = 2048  # psum chunk: 4 matmuls of 512 cols, one copy out

    with tile.TileContext(nc) as tc:
        with tc.tile_pool(name="const", bufs=1) as cpool, \
             tc.tile_pool(name="psum", bufs=2, space="PSUM") as ppool:
            xT_sb = cpool.tile([F_IN, XPAD], f16)
            W1_sb = cpool.tile([F_IN, H], f16)
            h0_sb = cpool.tile([H, XPAD], f16)
            nc.sync.dma_start(out=xT_sb[:], in_=xT_d[:])
            nc.sync.dma_start(out=W1_sb[:], in_=W1_d[:])

            def body():
                for ci, j0 in enumerate(range(0, XPAD, PCH)):
                    pw = min(PCH, XPAD - j0)
                    ps = ppool.tile([H, PCH], f32, tag="ps")
                    for j in range(j0, j0 + pw, XCH):
                        nc.tensor.matmul(out=ps[:, j - j0:j - j0 + XCH],
                                         lhsT=W1_sb[:], rhs=xT_sb[:, j:j + XCH],
                                         start=True, stop=True)
                    if ci % 2:
                        nc.scalar.copy(out=h0_sb[:, j0:j0 + pw], in_=ps[:, :pw])
                    else:
                        nc.vector.tensor_copy(out=h0_sb[:, j0:j0 + pw],
                                              in_=ps[:, :pw])
                nc.sync.dma_start(out=h0_d[:], in_=h0_sb[:])

            if loop_reps == 1:
                body()
            else:
                with tc.For_i(0, loop_reps, 1):
                    body()
    nc.compile()
    return nc


def build_stageB(slot_tot, Lg, nwg, goff, loop_reps=1):
    """Layer-1 weighted segment-sum over 16-wide fp16 messages, then
    z = relu(agg + 64*b1), z2 = z @ (kron(I8,W2)/64) -> fp16 [P, NWIN*C]."""
    nc = bacc.Bacc("TRN2", target_bir_lowering=False, debug=False,
                   num_devices=NCORES)
    f32, f16 = mybir.dt.float32, mybir.dt.float16
    msg_d = nc.dram_tensor("msg", [P, slot_tot * H], f16, kind="ExternalInput").ap()
    wn_d = nc.dram_tensor("wn", [P, slot_tot], f16, kind="ExternalInput").ap()
    W2b_d = nc.dram_tensor("W2b", [P, GW * C], f32, kind="ExternalInput").ap()
    b1c_d = nc.dram_tensor("b1c", [GW * H, 1], f32, kind="ExternalInput").ap()
    out_d = nc.dram_tensor("out", [P, NWIN * C], f16, kind="ExternalOutput").ap()

    maxblk = max(int(nwg[g]) * int(Lg[g]) for g in range(NGRP)) * H
    mult, add = mybir.AluOpType.mult, mybir.AluOpType.add

    with tile.TileContext(nc) as tc:
        with tc.tile_pool(name="const", bufs=1) as cpool, \
             tc.tile_pool(name="msgs", bufs=4) as pool, \
             tc.tile_pool(name="epi", bufs=3) as epool, \
             tc.tile_pool(name="psum", bufs=2, space="PSUM") as ppool:
            wn_sb = cpool.tile([P, slot_tot], f16)
            W2b_sb = cpool.tile([P, GW * C], f32)
            b1c_sb = cpool.tile([GW * H, 1], f32)
            ident = cpool.tile([P, P], f32)
            out_all = cpool.tile([P, NWIN * C], f16)
            nc.sync.dma_start(out=wn_sb[:], in_=wn_d[:])
            nc.sync.dma_start(out=W2b_sb[:], in_=W2b_d[:])
            nc.sync.dma_start(out=b1c_sb[:], in_=b1c_d[:])
            make_identity(nc, ident[:])

            def body():
                for g in range(NGRP):
                    nw, L, off = int(nwg[g]), int(Lg[g]), int(goff[g])
                    blk = nw * H * L
                    msg = pool.tile([P, maxblk], f16, tag="msg")
                    nc.sync.dma_start(out=msg[:, :blk],
                                      in_=msg_d[:, off * H:(off + nw * L) * H])
                    m4 = msg[:, :blk].rearrange("p (w f s) -> p w f s",
                                                w=nw, f=H)
                    wb = (wn_sb[:, off:off + nw * L]
                          .rearrange("p (w s) -> p w s", w=nw)
                          .unsqueeze(2).to_broadcast([P, nw, H, L]))
                    nc.vector.tensor_tensor(out=m4, in0=m4, in1=wb, op=mult)
                    h = L // 2
                    nc.vector.tensor_tensor(out=m4[:, :, :, :h],
                                            in0=m4[:, :, :, :h],
                                            in1=m4[:, :, :, h:2 * h], op=add)
                    q = h // 2
                    nc.vector.tensor_tensor(out=m4[:, :, :, :q],
                                            in0=m4[:, :, :, :q],
                                            in1=m4[:, :, :, q:2 * q], op=add)
                    agg = epool.tile([P, GW * H], f32, tag="agg")
                    a3 = agg[:, :nw * H].rearrange("p (w f) -> p w f", w=nw)
                    nc.vector.tensor_reduce(out=a3, in_=m4[:, :, :, :q],
                                            axis=mybir.AxisListType.X, op=add)
                    # transpose agg, then zT = relu(aggT + 64*b1) fused on Act
                    # (bias is per-partition [f] in the transposed layout);
                    # the 1/64 is folded into W2b
                    zT_p = ppool.tile([GW * H, P], f32, tag="zT")
                    nc.tensor.transpose(out=zT_p[:nw * H, :], in_=agg[:, :nw * H],
                                        identity=ident[:])
                    zT = epool.tile([GW * H, P], f32, tag="zTs")
                    nc.scalar.activation(out=zT[:nw * H, :], in_=zT_p[:nw * H, :],
                                         func=mybir.ActivationFunctionType.Relu,
                                         bias=b1c_sb[:nw * H, :])
                    o8 = ppool.tile([P, GW * C], f32, tag="o8")
                    nc.tensor.matmul(out=o8[:, :nw * C], lhsT=zT[:nw * H, :],
                                     rhs=W2b_sb[:nw * H, :nw * C],
                                     start=True, stop=True)
                    nc.scalar.copy(out=out_all[:, g * GW * C:g * GW * C + nw * C],
                                   in_=o8[:, :nw * C])
                nc.sync.dma_start(out=out_d[:], in_=out_all[:])

            if loop_reps == 1:
                body()
            else:
                with tc.For_i(0, loop_reps, 1):
                    body()
    nc.compile()
    return nc


def build_stageC(slot_tot, Lg, nwg, goff, loop_reps=1):
    """Layer-2 weighted segment-sum over 2-wide fp16 messages, then
    log_softmax(agg/64 + b2) -> f32 [P, NWIN*C]."""
    nc = bacc.Bacc("TRN2", target_bir_lowering=False, debug=False,
                   num_devices=NCORES)
    f32, f16 = mybir.dt.float32, mybir.dt.float16
    msg_d = nc.dram_tensor("msg", [P, slot_tot * C], f16, kind="ExternalInput").ap()
    wn_d = nc.dram_tensor("wn", [P, slot_tot], f16, kind="ExternalInput").ap()
    b2_d = nc.dram_tensor("b2", [P, C], f32, kind="ExternalInput").ap()
    out_d = nc.dram_tensor("out", [P, NWIN * C], f32, kind="ExternalOutput").ap()

    maxblk = max(int(nwg[g]) * int(Lg[g]) for g in range(NGRP)) * C
    mult, add = mybir.AluOpType.mult, mybir.AluOpType.add
    sub = mybir.AluOpType.subtract

    with tile.TileContext(nc) as tc:
        with tc.tile_pool(name="const", bufs=1) as cpool, \
             tc.tile_pool(name="msgs", bufs=4) as pool, \
             tc.tile_pool(name="epi", bufs=2) as epool:
            wn_sb = cpool.tile([P, slot_tot], f16)
            b2_sb = cpool.tile([P, C], f32)
            agg_all = cpool.tile([P, NWIN * C], f32)
            out_all = cpool.tile([P, NWIN * C], f32)
            nc.sync.dma_start(out=wn_sb[:], in_=wn_d[:])
            nc.sync.dma_start(out=b2_sb[:], in_=b2_d[:])

            def body():
                for g in range(NGRP):
                    nw, L, off = int(nwg[g]), int(Lg[g]), int(goff[g])
                    blk = nw * C * L
                    msg = pool.tile([P, maxblk], f16, tag="msg")
                    nc.sync.dma_start(out=msg[:, :blk],
                                      in_=msg_d[:, off * C:(off + nw * L) * C])
                    m4 = msg[:, :blk].rearrange("p (w f s) -> p w f s",
                                                w=nw, f=C)
                    wb = (wn_sb[:, off:off + nw * L]
                          .rearrange("p (w s) -> p w s", w=nw)
                          .unsqueeze(2).to_broadcast([P, nw, C, L]))
                    nc.vector.tensor_tensor(out=m4, in0=m4, in1=wb, op=mult)
                    h = L // 2
                    nc.vector.tensor_tensor(out=m4[:, :, :, :h],
                                            in0=m4[:, :, :, :h],
                                            in1=m4[:, :, :, h:2 * h], op=add)
                    q = h // 2
                    nc.vector.tensor_tensor(out=m4[:, :, :, :q],
                                            in0=m4[:, :, :, :q],
                                            in1=m4[:, :, :, q:2 * q], op=add)
                    a3 = agg_all[:, g * GW * C:g * GW * C + nw * C].rearrange(
                        "p (w c) -> p w c", w=nw)
                    nc.vector.tensor_reduce(out=a3, in_=m4[:, :, :, :q],
                                            axis=mybir.AxisListType.X, op=add)
                # batched epilogue (one Exp / one Ln per iteration: the Act
                # engine reloads its function table on every func switch,
                # 1.28 us each — per-group exp/ln cost 28 us/iter in tables)
                t = epool.tile([P, NWIN * C], f32, tag="t")
                a3 = agg_all[:].rearrange("p (w c) -> p w c", c=C)
                t3 = t[:].rearrange("p (w c) -> p w c", c=C)
                b2b = b2_sb[:].unsqueeze(1).to_broadcast([P, NWIN, C])
                nc.vector.scalar_tensor_tensor(out=t3, in0=a3,
                                               scalar=1.0 / WSCALE,
                                               in1=b2b, op0=mult, op1=add)
                rmax = epool.tile([P, NWIN], f32, tag="rmax")
                nc.vector.tensor_reduce(out=rmax[:], in_=t3,
                                        axis=mybir.AxisListType.X,
                                        op=mybir.AluOpType.max)
                rb = rmax[:].unsqueeze(2).to_broadcast([P, NWIN, C])
                nc.vector.tensor_tensor(out=t3, in0=t3, in1=rb, op=sub)
                e = epool.tile([P, NWIN * C], f32, tag="e")
                nc.scalar.activation(out=e[:], in_=t[:],
                                     func=mybir.ActivationFunctionType.Exp)
                se = epool.tile([P, NWIN], f32, tag="se")
                nc.vector.tensor_reduce(
                    out=se[:], in_=e[:].rearrange("p (w c) -> p w c", c=C),
                    axis=mybir.AxisListType.X, op=add)
                lse = epool.tile([P, NWIN], f32, tag="lse")
                nc.scalar.activation(out=lse[:], in_=se[:],
                                     func=mybir.ActivationFunctionType.Ln)
                ob = out_all[:].rearrange("p (w c) -> p w c", c=C)
                lb = lse[:].unsqueeze(2).to_broadcast([P, NWIN, C])
                nc.vector.tensor_tensor(out=ob, in0=t3, in1=lb, op=sub)
                nc.sync.dma_start(out=out_d[:], in_=out_all[:])

            if loop_reps == 1:
                body()
            else:
                with tc.For_i(0, loop_reps, 1):
                    body()
    nc.compile()
    return nc


# ----------------------------------------------------------------------------
# Full model
# ----------------------------------------------------------------------------

_CACHE = {}


def get_programs(g, loop_reps=1):
    key = (g["slot_tot"], tuple(g["Lg"]), loop_reps)
    if key not in _CACHE:
        kA = build_stageA(loop_reps)
        kB = build_stageB(g["slot_tot"], g["Lg"], g["nwg"], g["goff"], loop_reps)
        kC = build_stageC(g["slot_tot"], g["Lg"], g["nwg"], g["goff"], loop_reps)
        _CACHE[key] = (kA, kB, kC)
    return _CACHE[key]


def kernel(x, edge_index, edge_weight, W1, b1, W2, b2):
    x = np.asarray(x, np.float32)
    W1 = np.asarray(W1, np.float32)
    b1 = np.asarray(b1, np.float32)
    W2 = np.asarray(W2, np.float32)
    b2 = np.asarray(b2, np.float32)

    g = preprocess_graph(edge_index, edge_weight)
    kA, kB, kC = get_programs(g, 1)
    cores = list(range(NCORES))

    # ---- stage A: H0 = X @ W1 ----
    W1h = W1.astype(np.float16)
    inA = []
    for c in range(NCORES):
        xT = np.zeros((F_IN, XPAD), np.float16)
        xT[:, :DPC] = x[c * DPC:(c + 1) * DPC].T.astype(np.float16)
        inA.append({"xT": xT, "W1": W1h})
    rA = bass_utils.run_bass_kernel_spmd(kA, inA, core_ids=cores)
    h0_full = np.empty((N, H), np.float16)
    for c in range(NCORES):
        h0_full[c * DPC:(c + 1) * DPC] = rA.results[c]["h0"][:, :DPC].T

    # ---- host halo exchange 1 + stage B ----
    W2b = (np.kron(np.eye(GW, dtype=np.float32), W2) / WSCALE).astype(np.float32)
    b1c = np.tile(b1 * WSCALE, GW).astype(np.float32)[:, None]
    inB = []
    for c in range(NCORES):
        msg = gather_group_msgs(h0_full, g["sp"][c], g["Lg"], g["nwg"],
                                g["goff"], H)
        inB.append({"msg": msg, "wn": g["wn16"][c], "W2b": W2b, "b1c": b1c})
    rB = bass_utils.run_bass_kernel_spmd(kB, inB, core_ids=cores)
    z2_full = np.empty((N, C), np.float16)
    for c in range(NCORES):
        arr = rB.results[c]["out"].reshape(P, NWIN, C).transpose(1, 0, 2)
        arr = arr.reshape(DPC_PAD, C)
        order, _ = g["perms"][c]
        loc = np.empty((DPC, C), np.float16)
        loc[order] = arr[:DPC]
        z2_full[c * DPC:(c + 1) * DPC] = loc

    # ---- host halo exchange 2 + stage C ----
    b2r = np.broadcast_to(b2, (P, C)).astype(np.float32).copy()
    inC = []
    for c in range(NCORES):
        msg = gather_group_msgs(z2_full, g["sp"][c], g["Lg"], g["nwg"],
                                g["goff"], C)
        inC.append({"msg": msg, "wn": g["wn16"][c], "b2": b2r})
    rC = bass_utils.run_bass_kernel_spmd(kC, inC, core_ids=cores)

    out = np.empty((N, C), np.float32)
    for c in range(NCORES):
        arr = rC.results[c]["out"].reshape(P, NWIN, C).transpose(1, 0, 2)
        arr = arr.reshape(DPC_PAD, C)
        order, _ = g["perms"][c]
        loc = np.empty((DPC, C), np.float32)
        loc[order] = arr[:DPC]
        out[c * DPC:(c + 1) * DPC] = loc
    return out


# revision 16
# speedup vs baseline: 59.3863x; 1.1347x over previous
"""2-layer GCN (GCNConv -> ReLU -> GCNConv -> log_softmax) on 8 TRN2 NeuronCores.

Strategy (dest-sharded, per the halo-exchange hint):
  - Nodes (and incident edges, by destination) are partitioned across the 8
    cores: core c owns destination nodes [c*12500, (c+1)*12500).
  - gcn_norm coefficients are host-side graph preprocessing; the per-edge
    coefficient wn = dinv[src]*w*dinv[dst] is one fp16 scalar per edge
    (scaled by WSCALE=64 so small values stay in fp16 normal range; the
    scale is folded back into W2 / the bias path on device).
  - Key reordering vs the naive formulation: aggregation happens AFTER the
    dense transform (A@(X@W1) == (A@X)@W1 and A@(h@W2)), so layer-1
    messages are 16 floats wide (not 37) and layer-2 messages 2 wide.
  - Halo exchange (gather of remote source features) is materialized on the
    host as per-core, destination-ordered ELLPACK message streams in fp16
    (measured: device-side random-access gathers cost 25-200ns/edge, 10-100x
    slower than streaming).  Streams use GROUP-UNIFORM padding: windows of
    128 destinations (degree-sorted) grouped by GW=16; all windows in a
    group share one slot width, so each group is one set of big 4D DVE ops
    ([128, w, f, s]) instead of per-window ops.
  - Stage A (device): H0 = X @ W1 (fp16 matmuls; fp32 runs the PE at 1/4
    rate), node-partitioned PSUM tiles so the PSUM->SBUF copies are cheap.
  - Host: gather H0 rows into layer-1 message streams.
  - Stage B (device): weighted segment-sum (DVE mult at 2x + fp16 add-tree
    + reduce; tensor_reduce is 1x-only so the tree halves its input twice),
    zT = relu(aggT + 64*b1) fused on Act after a PE transpose, z2 = z @
    (kron(I8,W2)/64) via block-diagonal matmul -> fp16 z2 shard.
  - Host: gather z2 rows into layer-2 message streams.
  - Stage C (device): weighted segment-sum of 2-wide messages, then
    log_softmax via Softplus only (one Act table; Exp/Ln table reloads cost
    1.28us each): out0 = -softplus(t1-t0), out1 = -softplus(t0-t1).
  - All feature arithmetic (FLOPs) happens on device; the host only moves /
    permutes bytes and preprocesses graph coefficients & weight layouts.
"""

import sys

sys.path.insert(0, "/opt/trn_rl_repo")

import numpy as np

from concourse import bass, mybir, bacc
import concourse.tile as tile
from concourse import bass_utils
from concourse.masks import make_identity

N = 100_000
NCORES = 8
DPC = N // NCORES            # 12500 dests per core
P = 128                      # partitions
NWIN = (DPC + P - 1) // P    # 98 windows of 128 dests
DPC_PAD = NWIN * P           # 12544

F_IN = 37
H = 16
C = 2

GW = 16                       # windows per aggregation group
NGRP = (NWIN + GW - 1) // GW  # 7 (6 full + 1 group of 2 windows)
EPB = 8                       # windows per epilogue sub-block (EPB*H == P)
WSCALE = 64.0                 # fp16 range scaling for wn


# ----------------------------------------------------------------------------
# Host-side graph preprocessing (indices / coefficients only - no feature math)
# ----------------------------------------------------------------------------

def preprocess_graph(edge_index, edge_weight):
    row = np.asarray(edge_index[0]).astype(np.int64)
    col = np.asarray(edge_index[1]).astype(np.int64)
    w = np.asarray(edge_weight).astype(np.float32)

    loop = np.arange(N, dtype=np.int64)
    row = np.concatenate([row, loop])
    col = np.concatenate([col, loop])
    w = np.concatenate([w, np.ones(N, np.float32)])

    deg = np.bincount(col, weights=w.astype(np.float64), minlength=N)
    dinv = np.where(deg > 0, 1.0 / np.sqrt(deg), 0.0).astype(np.float32)
    wn = dinv[row] * w * dinv[col]  # [E+N] f32

    core = col // DPC
    shards = []
    for c in range(NCORES):
        m = core == c
        shards.append((row[m], col[m] - c * DPC, wn[m]))

    # per-core degree-sorted dest permutation (uniform geometry across cores)
    perms, counts_sorted = [], []
    for c in range(NCORES):
        _, ld, _ = shards[c]
        cnt = np.bincount(ld, minlength=DPC)
        order = np.argsort(-cnt, kind="stable")       # rank -> local dest
        permpos = np.empty(DPC, np.int64)
        permpos[order] = np.arange(DPC)               # local dest -> rank
        perms.append((order, permpos))
        cs = np.zeros(DPC_PAD, np.int64)
        cs[:DPC] = cnt[order]
        counts_sorted.append(cs)

    # group-uniform slot widths: max count within each group of GW windows,
    # across all cores; padded to a multiple of 4 for the 2-level add-tree
    cnt_all = np.stack(counts_sorted)                 # [8, 12544]
    wmax = cnt_all.reshape(NCORES, NWIN, P).max(axis=(0, 2))  # per-window max
    Lg = np.zeros(NGRP, np.int64)
    nwg = np.zeros(NGRP, np.int64)
    for g in range(NGRP):
        w0, w1 = GW * g, min(GW * (g + 1), NWIN)
        nwg[g] = w1 - w0
        Lg[g] = max(int(wmax[w0:w1].max()), 1)
    Lg = ((Lg + 3) // 4) * 4
    Lwin = Lg[np.arange(NWIN) // GW]                  # per-window width
    slotoff = np.concatenate([[0], np.cumsum(Lwin)])
    slot_tot = int(slotoff[-1])
    goff = slotoff[GW * np.arange(NGRP)]

    # per-core slot assignment: [128, slot_tot] arrays of src node id and wn
    sp_all, wn_all = [], []
    for c in range(NCORES):
        src, ld, wnc = shards[c]
        _, permpos = perms[c]
        q = permpos[ld]                                # rank of each edge's dest
        sort = np.argsort(q, kind="stable")
        qs, srcs, wns = q[sort], src[sort], wnc[sort]
        cnt = np.bincount(qs, minlength=DPC_PAD)
        starts = np.concatenate([[0], np.cumsum(cnt)])[:-1]
        slot = np.arange(len(qs)) - starts[qs]
        wi = qs // P
        colidx = slotoff[wi] + slot
        pi = qs % P
        sp = np.zeros((P, slot_tot), np.int64)
        wa = np.zeros((P, slot_tot), np.float16)
        sp[pi, colidx] = srcs
        wa[pi, colidx] = (wns * WSCALE).astype(np.float16)
        sp_all.append(sp)
        wn_all.append(wa)

    return {
        "Lg": Lg, "nwg": nwg, "goff": goff, "slot_tot": slot_tot,
        "sp": sp_all, "wn16": wn_all, "perms": perms,
    }


def gather_group_msgs(vals, sp, Lg, nwg, goff, F):
    """vals [N, F] fp16, sp [P, slot_tot] -> msg stream [P, slot_tot*F] fp16
    with per-group layout [w, f, s] (s innermost)."""
    a = vals[sp]                                      # [P, slot_tot, F]
    slot_tot = sp.shape[1]
    out = np.empty((P, slot_tot * F), vals.dtype)
    for g in range(len(Lg)):
        nw, L, off = int(nwg[g]), int(Lg[g]), int(goff[g])
        seg = a[:, off:off + nw * L, :].reshape(P, nw, L, F)
        out[:, off * F:(off + nw * L) * F] = (
            seg.transpose(0, 1, 3, 2).reshape(P, nw * L * F))
    return out


def unpermute_out(arr, order, dtype):
    """Device layout [P, NWIN*C'] (node w*128+p at [p, w]) -> [DPC, C']."""
    c2 = arr.shape[1] // NWIN
    a = arr.reshape(P, NWIN, c2).transpose(1, 0, 2).reshape(DPC_PAD, c2)
    loc = np.empty((DPC, c2), dtype)
    loc[order] = a[:DPC]
    return loc


# ----------------------------------------------------------------------------
# Device programs
# ----------------------------------------------------------------------------

def build_stageA(loop_reps=1):
    """H0 = X @ W1 for the core's node shard, node-partitioned.
    xT [37, DPC_PAD] fp16 -> h0 [P, NWIN*H] fp16 (node w*128+p at [p, w])."""
    nc = bacc.Bacc("TRN2", target_bir_lowering=False, debug=False,
                   num_devices=NCORES)
    f32, f16 = mybir.dt.float32, mybir.dt.float16
    xT_d = nc.dram_tensor("xT", [F_IN, DPC_PAD], f16, kind="ExternalInput").ap()
    W1_d = nc.dram_tensor("W1", [F_IN, H], f16, kind="ExternalInput").ap()
    h0_d = nc.dram_tensor("h0", [P, NWIN * H], f16, kind="ExternalOutput").ap()

    AB = 8  # window-chunks per psum tile ([P, AB*H] = one bank) / copy batch

    with tile.TileContext(nc) as tc:
        with tc.tile_pool(name="const", bufs=1) as cpool, \
             tc.tile_pool(name="psum", bufs=4, space="PSUM") as ppool:
            xT_sb = cpool.tile([F_IN, DPC_PAD], f16)
            W1_sb = cpool.tile([F_IN, H], f16)
            h0_sb = cpool.tile([P, NWIN * H], f16)
            nc.sync.dma_start(out=xT_sb[:], in_=xT_d[:])
            nc.sync.dma_start(out=W1_sb[:], in_=W1_d[:])

            def body():
                for ci, w0 in enumerate(range(0, NWIN, AB)):
                    nb = min(AB, NWIN - w0)
                    pb = ppool.tile([P, AB * H], f32, tag="pb")
                    for j in range(nb):
                        w = w0 + j
                        nc.tensor.matmul(out=pb[:, j * H:(j + 1) * H],
                                         lhsT=xT_sb[:, w * P:(w + 1) * P],
                                         rhs=W1_sb[:], start=True, stop=True)
                    dst = h0_sb[:, w0 * H:(w0 + nb) * H]
                    if ci % 2:
                        nc.scalar.copy(out=dst, in_=pb[:, :nb * H])
                    else:
                        nc.vector.tensor_copy(out=dst, in_=pb[:, :nb * H])
                nc.sync.dma_start(out=h0_d[:], in_=h0_sb[:])

            if loop_reps == 1:
                body()
            else:
                with tc.For_i(0, loop_reps, 1, staggered_reset=True):
                    body()
    nc.compile()
    return nc


def build_stageB(slot_tot, Lg, nwg, goff, loop_reps=1):
    """Layer-1 weighted segment-sum over 16-wide fp16 messages, then
    z = relu(agg + 64*b1), z2 = z @ (kron(I8,W2)/64) -> fp16 [P, NWIN*C]."""
    nc = bacc.Bacc("TRN2", target_bir_lowering=False, debug=False,
                   num_devices=NCORES)
    f32, f16 = mybir.dt.float32, mybir.dt.float16
    msg_d = nc.dram_tensor("msg", [P, slot_tot * H], f16, kind="ExternalInput").ap()
    wn_d = nc.dram_tensor("wn", [P, slot_tot], f16, kind="ExternalInput").ap()
    W2b_d = nc.dram_tensor("W2b", [P, EPB * C], f32, kind="ExternalInput").ap()
    b1c_d = nc.dram_tensor("b1c", [EPB * H, 1], f32, kind="ExternalInput").ap()
    out_d = nc.dram_tensor("out", [P, NWIN * C], f16, kind="ExternalOutput").ap()

    maxblk = max(int(nwg[g]) * int(Lg[g]) for g in range(NGRP)) * H
    mult, add = mybir.AluOpType.mult, mybir.AluOpType.add

    with tile.TileContext(nc) as tc:
        with tc.tile_pool(name="const", bufs=1) as cpool, \
             tc.tile_pool(name="msgs", bufs=3) as pool, \
             tc.tile_pool(name="epi", bufs=3) as epool, \
             tc.tile_pool(name="psum", bufs=2, space="PSUM") as ppool:
            wn_sb = cpool.tile([P, slot_tot], f16)
            W2b_sb = cpool.tile([P, EPB * C], f32)
            b1c_sb = cpool.tile([EPB * H, 1], f32)
            ident = cpool.tile([P, P], f32)
            out_all = cpool.tile([P, NWIN * C], f16)
            nc.sync.dma_start(out=wn_sb[:], in_=wn_d[:])
            nc.sync.dma_start(out=W2b_sb[:], in_=W2b_d[:])
            nc.sync.dma_start(out=b1c_sb[:], in_=b1c_d[:])
            make_identity(nc, ident[:])

            def body():
                for g in reversed(range(NGRP)):
                    nw, L, off = int(nwg[g]), int(Lg[g]), int(goff[g])
                    blk = nw * H * L
                    msg = pool.tile([P, maxblk], f16, tag="msg")
                    nc.sync.dma_start(out=msg[:, :blk],
                                      in_=msg_d[:, off * H:(off + nw * L) * H])
                    m4 = msg[:, :blk].rearrange("p (w f s) -> p w f s",
                                                w=nw, f=H)
                    wb = (wn_sb[:, off:off + nw * L]
                          .rearrange("p (w s) -> p w s", w=nw)
                          .unsqueeze(2).to_broadcast([P, nw, H, L]))
                    nc.vector.tensor_tensor(out=m4, in0=m4, in1=wb, op=mult)
                    h = L // 2
                    nc.vector.tensor_tensor(out=m4[:, :, :, :h],
                                            in0=m4[:, :, :, :h],
                                            in1=m4[:, :, :, h:2 * h], op=add)
                    q = h // 2
                    nc.vector.tensor_tensor(out=m4[:, :, :, :q],
                                            in0=m4[:, :, :, :q],
                                            in1=m4[:, :, :, q:2 * q], op=add)
                    agg = epool.tile([P, GW * H], f32, tag="agg")
                    a3 = agg[:, :nw * H].rearrange("p (w f) -> p w f", w=nw)
                    nc.vector.tensor_reduce(out=a3, in_=m4[:, :, :, :q],
                                            axis=mybir.AxisListType.X, op=add)
                    # epilogue in EPB-window sub-blocks (EPB*H = 128):
                    # transpose agg -> zT = relu(aggT + 64*b1) fused on Act
                    # (bias is per-partition in the transposed layout);
                    # the 1/64 is folded into W2b
                    for j in range(0, nw, EPB):
                        nb = min(EPB, nw - j)
                        zT_p = ppool.tile([EPB * H, P], f32, tag="zT")
                        nc.tensor.transpose(
                            out=zT_p[:nb * H, :],
                            in_=agg[:, j * H:(j + nb) * H],
                            identity=ident[:])
                        zT = epool.tile([EPB * H, P], f32, tag="zTs")
                        nc.scalar.activation(
                            out=zT[:nb * H, :], in_=zT_p[:nb * H, :],
                            func=mybir.ActivationFunctionType.Relu,
                            bias=b1c_sb[:nb * H, :])
                        o8 = ppool.tile([P, EPB * C], f32, tag="o8")
                        nc.tensor.matmul(out=o8[:, :nb * C],
                                         lhsT=zT[:nb * H, :],
                                         rhs=W2b_sb[:nb * H, :nb * C],
                                         start=True, stop=True)
                        col = (GW * g + j) * C
                        nc.scalar.copy(out=out_all[:, col:col + nb * C],
                                       in_=o8[:, :nb * C])
                nc.sync.dma_start(out=out_d[:], in_=out_all[:])

            if loop_reps == 1:
                body()
            else:
                with tc.For_i(0, loop_reps, 1, staggered_reset=True):
                    body()
    nc.compile()
    return nc


def build_stageC(slot_tot, Lg, nwg, goff, loop_reps=1):
    """Layer-2 weighted segment-sum over 2-wide fp16 messages, then
    log_softmax(agg/64 + b2) via Softplus -> f32 [P, NWIN*C]."""
    nc = bacc.Bacc("TRN2", target_bir_lowering=False, debug=False,
                   num_devices=NCORES)
    f32, f16 = mybir.dt.float32, mybir.dt.float16
    msg_d = nc.dram_tensor("msg", [P, slot_tot * C], f16, kind="ExternalInput").ap()
    wn_d = nc.dram_tensor("wn", [P, slot_tot], f16, kind="ExternalInput").ap()
    b2_d = nc.dram_tensor("b2", [P, C], f32, kind="ExternalInput").ap()
    out_d = nc.dram_tensor("out", [P, NWIN * C], f32, kind="ExternalOutput").ap()

    maxblk = max(int(nwg[g]) * int(Lg[g]) for g in range(NGRP)) * C
    mult, add = mybir.AluOpType.mult, mybir.AluOpType.add
    sub = mybir.AluOpType.subtract

    with tile.TileContext(nc) as tc:
        with tc.tile_pool(name="const", bufs=1) as cpool, \
             tc.tile_pool(name="msgs", bufs=3) as pool, \
             tc.tile_pool(name="epi", bufs=2) as epool:
            wn_sb = cpool.tile([P, slot_tot], f16)
            b2_sb = cpool.tile([P, C], f32)
            agg_all = cpool.tile([P, NWIN * C], f32)
            out_all = cpool.tile([P, NWIN * C], f32)
            nc.sync.dma_start(out=wn_sb[:], in_=wn_d[:])
            nc.sync.dma_start(out=b2_sb[:], in_=b2_d[:])

            def body():
                for g in reversed(range(NGRP)):
                    nw, L, off = int(nwg[g]), int(Lg[g]), int(goff[g])
                    blk = nw * C * L
                    msg = pool.tile([P, maxblk], f16, tag="msg")
                    nc.sync.dma_start(out=msg[:, :blk],
                                      in_=msg_d[:, off * C:(off + nw * L) * C])
                    m4 = msg[:, :blk].rearrange("p (w f s) -> p w f s",
                                                w=nw, f=C)
                    wb = (wn_sb[:, off:off + nw * L]
                          .rearrange("p (w s) -> p w s", w=nw)
                          .unsqueeze(2).to_broadcast([P, nw, C, L]))
                    nc.vector.tensor_tensor(out=m4, in0=m4, in1=wb, op=mult)
                    h = L // 2
                    nc.vector.tensor_tensor(out=m4[:, :, :, :h],
                                            in0=m4[:, :, :, :h],
                                            in1=m4[:, :, :, h:2 * h], op=add)
                    q = h // 2
                    nc.vector.tensor_tensor(out=m4[:, :, :, :q],
                                            in0=m4[:, :, :, :q],
                                            in1=m4[:, :, :, q:2 * q], op=add)
                    a3 = agg_all[:, GW * g * C:(GW * g + nw) * C].rearrange(
                        "p (w c) -> p w c", w=nw)
                    nc.vector.tensor_reduce(out=a3, in_=m4[:, :, :, :q],
                                            axis=mybir.AxisListType.X, op=add)
                # batched epilogue (one Exp / one Ln per iteration: the Act
                # engine reloads its function table on every func switch,
                # 1.28 us each)
                t = epool.tile([P, NWIN * C], f32, tag="t")
                a3 = agg_all[:].rearrange("p (w c) -> p w c", c=C)
                t3 = t[:].rearrange("p (w c) -> p w c", c=C)
                b2b = b2_sb[:].unsqueeze(1).to_broadcast([P, NWIN, C])
                nc.vector.scalar_tensor_tensor(out=t3, in0=a3,
                                               scalar=1.0 / WSCALE,
                                               in1=b2b, op0=mult, op1=add)
                rmax = epool.tile([P, NWIN], f32, tag="rmax")
                nc.vector.tensor_reduce(out=rmax[:], in_=t3,
                                        axis=mybir.AxisListType.X,
                                        op=mybir.AluOpType.max)
                rb = rmax[:].unsqueeze(2).to_broadcast([P, NWIN, C])
                nc.vector.tensor_tensor(out=t3, in0=t3, in1=rb, op=sub)
                e = epool.tile([P, NWIN * C], f32, tag="e")
                nc.scalar.activation(out=e[:], in_=t[:],
                                     func=mybir.ActivationFunctionType.Exp)
                se = epool.tile([P, NWIN], f32, tag="se")
                nc.vector.tensor_reduce(
                    out=se[:], in_=e[:].rearrange("p (w c) -> p w c", c=C),
                    axis=mybir.AxisListType.X, op=add)
                lse = epool.tile([P, NWIN], f32, tag="lse")
                nc.scalar.activation(out=lse[:], in_=se[:],
                                     func=mybir.ActivationFunctionType.Ln)
                ob = out_all[:].rearrange("p (w c) -> p w c", c=C)
                lb = lse[:].unsqueeze(2).to_broadcast([P, NWIN, C])
                nc.vector.tensor_tensor(out=ob, in0=t3, in1=lb, op=sub)
                nc.sync.dma_start(out=out_d[:], in_=out_all[:])

            if loop_reps == 1:
                body()
            else:
                with tc.For_i(0, loop_reps, 1, staggered_reset=True):
                    body()
    nc.compile()
    return nc


# ----------------------------------------------------------------------------
# Full model
# ----------------------------------------------------------------------------

_CACHE = {}


def get_programs(g, loop_reps=1):
    key = (g["slot_tot"], tuple(g["Lg"]), loop_reps)
    if key not in _CACHE:
        kA = build_stageA(loop_reps)
        kB = build_stageB(g["slot_tot"], g["Lg"], g["nwg"], g["goff"], loop_reps)
        kC = build_stageC(g["slot_tot"], g["Lg"], g["nwg"], g["goff"], loop_reps)
        _CACHE[key] = (kA, kB, kC)
    return _CACHE[key]


def kernel(x, edge_index, edge_weight, W1, b1, W2, b2):
    x = np.asarray(x, np.float32)
    W1 = np.asarray(W1, np.float32)
    b1 = np.asarray(b1, np.float32)
    W2 = np.asarray(W2, np.float32)
    b2 = np.asarray(b2, np.float32)

    g = preprocess_graph(edge_index, edge_weight)
    kA, kB, kC = get_programs(g, 1)
    cores = list(range(NCORES))

    # ---- stage A: H0 = X @ W1 (node-partitioned output) ----
    W1h = W1.astype(np.float16)
    inA = []
    for c in range(NCORES):
        xT = np.zeros((F_IN, DPC_PAD), np.float16)
        xT[:, :DPC] = x[c * DPC:(c + 1) * DPC].T.astype(np.float16)
        inA.append({"xT": xT, "W1": W1h})
    rA = bass_utils.run_bass_kernel_spmd(kA, inA, core_ids=cores)
    h0_full = np.empty((N, H), np.float16)
    for c in range(NCORES):
        arr = rA.results[c]["h0"].reshape(P, NWIN, H).transpose(1, 0, 2)
        h0_full[c * DPC:(c + 1) * DPC] = arr.reshape(DPC_PAD, H)[:DPC]

    # ---- host halo exchange 1 + stage B ----
    W2b = (np.kron(np.eye(EPB, dtype=np.float32), W2) / WSCALE).astype(np.float32)
    b1c = np.tile(b1 * WSCALE, EPB).astype(np.float32)[:, None]
    inB = []
    for c in range(NCORES):
        msg = gather_group_msgs(h0_full, g["sp"][c], g["Lg"], g["nwg"],
                                g["goff"], H)
        inB.append({"msg": msg, "wn": g["wn16"][c], "W2b": W2b, "b1c": b1c})
    rB = bass_utils.run_bass_kernel_spmd(kB, inB, core_ids=cores)
    z2_full = np.empty((N, C), np.float16)
    for c in range(NCORES):
        order, _ = g["perms"][c]
        z2_full[c * DPC:(c + 1) * DPC] = unpermute_out(
            rB.results[c]["out"], order, np.float16)

    # ---- host halo exchange 2 + stage C ----
    b2r = np.broadcast_to(b2, (P, C)).astype(np.float32).copy()
    inC = []
    for c in range(NCORES):
        msg = gather_group_msgs(z2_full, g["sp"][c], g["Lg"], g["nwg"],
                                g["goff"], C)
        inC.append({"msg": msg, "wn": g["wn16"][c], "b2": b2r})
    rC = bass_utils.run_bass_kernel_spmd(kC, inC, core_ids=cores)

    out = np.empty((N, C), np.float32)
    for c in range(NCORES):
        order, _ = g["perms"][c]
        out[c * DPC:(c + 1) * DPC] = unpermute_out(
            rC.results[c]["out"], order, np.float32)
    return out


# revision 21
# speedup vs baseline: 61.3779x; 1.0335x over previous
"""2-layer GCN (GCNConv -> ReLU -> GCNConv -> log_softmax) on 8 TRN2 NeuronCores.

Strategy (dest-sharded, per the halo-exchange hint):
  - Nodes (and incident edges, by destination) are partitioned across the 8
    cores: core c owns destination nodes [c*12500, (c+1)*12500).
  - gcn_norm coefficients are host-side graph preprocessing; the per-edge
    coefficient wn = dinv[src]*w*dinv[dst] is one fp16 scalar per edge
    (scaled by WSCALE=64 so small values stay in fp16 normal range; the
    scale is folded back into W2 / the bias path on device).
  - Key reordering vs the naive formulation: aggregation happens AFTER the
    dense transform (A@(X@W1) == (A@X)@W1 and A@(h@W2)), so layer-1
    messages are 16 floats wide (not 37) and layer-2 messages 2 wide.
  - Halo exchange (gather of remote source features) is materialized on the
    host as per-core, destination-ordered ELLPACK message streams in fp16
    (measured: device-side random-access gathers cost 25-200ns/edge, 10-100x
    slower than streaming).  Streams use GROUP-UNIFORM padding: windows of
    128 destinations (degree-sorted) grouped by GW=16; all windows in a
    group share one slot width, so each group is one set of big 4D DVE ops
    ([128, w, f, s]) instead of per-window ops.
  - Stage A (device): H0 = X @ W1 (fp16 matmuls; fp32 runs the PE at 1/4
    rate), node-partitioned PSUM tiles so the PSUM->SBUF copies are cheap.
  - Host: gather H0 rows into layer-1 message streams.
  - Stage B (device): weighted segment-sum (DVE mult at 2x + fp16 add-tree
    + reduce; tensor_reduce is 1x-only so the tree halves its input twice),
    zT = relu(aggT + 64*b1) fused on Act after a PE transpose, z2 = z @
    (kron(I8,W2)/64) via block-diagonal matmul -> fp16 z2 shard.
  - Host: gather z2 rows into layer-2 message streams.
  - Stage C (device): weighted segment-sum of 2-wide messages, then a
    batched log_softmax epilogue (Exp+Ln share one activation table so no
    per-call 1.28us table reloads).
  - All feature arithmetic (FLOPs) happens on device; the host only moves /
    permutes bytes and preprocesses graph coefficients & weight layouts.
"""

import sys

sys.path.insert(0, "/opt/trn_rl_repo")

import numpy as np

from concourse import bass, mybir, bacc
import concourse.tile as tile
from concourse import bass_utils
from concourse.hw_specs import get_activation_tables
from concourse.masks import make_identity


class _BaccSharedExpLn(bacc.Bacc):
    """Bacc whose activation-table placement sees Exp/Ln only in the one
    table that contains both ('natural_log_exp_and_others'), so alternating
    Exp/Ln does not reload tables (1.28 us per reload).  Table indices are
    unchanged - the runtime still loads the real canonical table."""

    def insert_act_table_loads(self):
        has_activation = any(
            isinstance(i, mybir.InstActivation)
            for b in self.main_func.blocks
            for i in b.instructions
        )
        if not has_activation:
            return
        exp = mybir.ActivationFunctionType.Exp
        ln = mybir.ActivationFunctionType.Ln
        shared = "natural_log_exp_and_others"
        tables = [
            (name, s if name == shared else (s - {exp, ln}))
            for name, s in get_activation_tables(self.m.arch).items()
        ]
        bacc._bass_rust.insert_act_table_loads(self, tables)

N = 100_000
NCORES = 8
DPC = N // NCORES            # 12500 dests per core
P = 128                      # partitions
NWIN = (DPC + P - 1) // P    # 98 windows of 128 dests
DPC_PAD = NWIN * P           # 12544

F_IN = 37
H = 16
C = 2

GW = 16                       # windows per aggregation group
NGRP = (NWIN + GW - 1) // GW  # 7 (6 full + 1 group of 2 windows)
EPB = 8                       # windows per epilogue sub-block (EPB*H == P)
WSCALE = 64.0                 # fp16 range scaling for wn


# ----------------------------------------------------------------------------
# Host-side graph preprocessing (indices / coefficients only - no feature math)
# ----------------------------------------------------------------------------

def preprocess_graph(edge_index, edge_weight):
    row = np.asarray(edge_index[0]).astype(np.int64)
    col = np.asarray(edge_index[1]).astype(np.int64)
    w = np.asarray(edge_weight).astype(np.float32)

    loop = np.arange(N, dtype=np.int64)
    row = np.concatenate([row, loop])
    col = np.concatenate([col, loop])
    w = np.concatenate([w, np.ones(N, np.float32)])

    deg = np.bincount(col, weights=w.astype(np.float64), minlength=N)
    dinv = np.where(deg > 0, 1.0 / np.sqrt(deg), 0.0).astype(np.float32)
    wn = dinv[row] * w * dinv[col]  # [E+N] f32

    core = col // DPC
    shards = []
    for c in range(NCORES):
        m = core == c
        shards.append((row[m], col[m] - c * DPC, wn[m]))

    # per-core degree-sorted dest permutation (uniform geometry across cores)
    perms, counts_sorted = [], []
    for c in range(NCORES):
        _, ld, _ = shards[c]
        cnt = np.bincount(ld, minlength=DPC)
        order = np.argsort(-cnt, kind="stable")       # rank -> local dest
        permpos = np.empty(DPC, np.int64)
        permpos[order] = np.arange(DPC)               # local dest -> rank
        perms.append((order, permpos))
        cs = np.zeros(DPC_PAD, np.int64)
        cs[:DPC] = cnt[order]
        counts_sorted.append(cs)

    # group-uniform slot widths: max count within each group of GW windows,
    # across all cores; padded to a multiple of 4 for the 2-level add-tree
    cnt_all = np.stack(counts_sorted)                 # [8, 12544]
    wmax = cnt_all.reshape(NCORES, NWIN, P).max(axis=(0, 2))  # per-window max
    Lg = np.zeros(NGRP, np.int64)
    nwg = np.zeros(NGRP, np.int64)
    for g in range(NGRP):
        w0, w1 = GW * g, min(GW * (g + 1), NWIN)
        nwg[g] = w1 - w0
        Lg[g] = max(int(wmax[w0:w1].max()), 1)
    Lg = ((Lg + 3) // 4) * 4
    Lwin = Lg[np.arange(NWIN) // GW]                  # per-window width
    slotoff = np.concatenate([[0], np.cumsum(Lwin)])
    slot_tot = int(slotoff[-1])
    goff = slotoff[GW * np.arange(NGRP)]

    # per-core slot assignment: [128, slot_tot] arrays of src node id and wn
    sp_all, wn_all = [], []
    for c in range(NCORES):
        src, ld, wnc = shards[c]
        _, permpos = perms[c]
        q = permpos[ld]                                # rank of each edge's dest
        sort = np.argsort(q, kind="stable")
        qs, srcs, wns = q[sort], src[sort], wnc[sort]
        cnt = np.bincount(qs, minlength=DPC_PAD)
        starts = np.concatenate([[0], np.cumsum(cnt)])[:-1]
        slot = np.arange(len(qs)) - starts[qs]
        wi = qs // P
        colidx = slotoff[wi] + slot
        pi = qs % P
        sp = np.zeros((P, slot_tot), np.int64)
        wa = np.zeros((P, slot_tot), np.float16)
        sp[pi, colidx] = srcs
        wa[pi, colidx] = (wns * WSCALE).astype(np.float16)
        sp_all.append(sp)
        wn_all.append(wa)

    return {
        "Lg": Lg, "nwg": nwg, "goff": goff, "slot_tot": slot_tot,
        "sp": sp_all, "wn16": wn_all, "perms": perms,
    }


def gather_group_msgs(vals, sp, Lg, nwg, goff, F):
    """vals [N, F] fp16, sp [P, slot_tot] -> msg stream [P, slot_tot*F] fp16
    with per-group layout [w, f, s] (s innermost)."""
    a = vals[sp]                                      # [P, slot_tot, F]
    slot_tot = sp.shape[1]
    out = np.empty((P, slot_tot * F), vals.dtype)
    for g in range(len(Lg)):
        nw, L, off = int(nwg[g]), int(Lg[g]), int(goff[g])
        seg = a[:, off:off + nw * L, :].reshape(P, nw, L, F)
        out[:, off * F:(off + nw * L) * F] = (
            seg.transpose(0, 1, 3, 2).reshape(P, nw * L * F))
    return out


def unpermute_out(arr, order, dtype):
    """Device layout [P, NWIN*C'] (node w*128+p at [p, w]) -> [DPC, C']."""
    c2 = arr.shape[1] // NWIN
    a = arr.reshape(P, NWIN, c2).transpose(1, 0, 2).reshape(DPC_PAD, c2)
    loc = np.empty((DPC, c2), dtype)
    loc[order] = a[:DPC]
    return loc


# ----------------------------------------------------------------------------
# Device programs
# ----------------------------------------------------------------------------

def build_stageA(loop_reps=1):
    """H0 = X @ W1 for the core's node shard, node-partitioned.
    xT [37, DPC_PAD] fp16 -> h0 [P, NWIN*H] fp16 (node w*128+p at [p, w])."""
    nc = bacc.Bacc("TRN2", target_bir_lowering=False, debug=False,
                   num_devices=NCORES)
    f32, f16 = mybir.dt.float32, mybir.dt.float16
    xT_d = nc.dram_tensor("xT", [F_IN, DPC_PAD], f16, kind="ExternalInput").ap()
    W1_d = nc.dram_tensor("W1", [F_IN, H], f16, kind="ExternalInput").ap()
    h0_d = nc.dram_tensor("h0", [P, NWIN * H], f16, kind="ExternalOutput").ap()

    AB = 8  # window-chunks per psum tile ([P, AB*H] = one bank) / copy batch

    with tile.TileContext(nc) as tc:
        with tc.tile_pool(name="const", bufs=1) as cpool, \
             tc.tile_pool(name="psum", bufs=4, space="PSUM") as ppool:
            xT_sb = cpool.tile([F_IN, DPC_PAD], f16)
            W1_sb = cpool.tile([F_IN, H], f16)
            h0_sb = cpool.tile([P, NWIN * H], f16)
            nc.sync.dma_start(out=xT_sb[:], in_=xT_d[:])
            nc.sync.dma_start(out=W1_sb[:], in_=W1_d[:])

            def body():
                for ci, w0 in enumerate(range(0, NWIN, AB)):
                    nb = min(AB, NWIN - w0)
                    pb = ppool.tile([P, AB * H], f32, tag="pb")
                    for j in range(nb):
                        w = w0 + j
                        nc.tensor.matmul(out=pb[:, j * H:(j + 1) * H],
                                         lhsT=xT_sb[:, w * P:(w + 1) * P],
                                         rhs=W1_sb[:], start=True, stop=True)
                    dst = h0_sb[:, w0 * H:(w0 + nb) * H]
                    if ci % 2:
                        nc.scalar.copy(out=dst, in_=pb[:, :nb * H])
                    else:
                        nc.vector.tensor_copy(out=dst, in_=pb[:, :nb * H])
                nc.scalar.dma_start(out=h0_d[:], in_=h0_sb[:])

            if loop_reps == 1:
                body()
            else:
                with tc.For_i(0, loop_reps, 1, staggered_reset=True):
                    body()
    nc.compile()
    return nc


def build_stageB(slot_tot, Lg, nwg, goff, loop_reps=1):
    """Layer-1 weighted segment-sum over 16-wide fp16 messages, then
    z = relu(agg + 64*b1), z2 = z @ (kron(I8,W2)/64) -> fp16 [P, NWIN*C]."""
    nc = bacc.Bacc("TRN2", target_bir_lowering=False, debug=False,
                   num_devices=NCORES)
    f32, f16 = mybir.dt.float32, mybir.dt.float16
    msg_d = nc.dram_tensor("msg", [P, slot_tot * H], f16, kind="ExternalInput").ap()
    wn_d = nc.dram_tensor("wn", [P, slot_tot], f16, kind="ExternalInput").ap()
    W2b_d = nc.dram_tensor("W2b", [P, EPB * C], f32, kind="ExternalInput").ap()
    b1c_d = nc.dram_tensor("b1c", [EPB * H, 1], f32, kind="ExternalInput").ap()
    out_d = nc.dram_tensor("out", [P, NWIN * C], f16, kind="ExternalOutput").ap()

    maxblk = max(int(nwg[g]) * int(Lg[g]) for g in range(NGRP)) * H
    mult, add = mybir.AluOpType.mult, mybir.AluOpType.add

    with tile.TileContext(nc) as tc:
        with tc.tile_pool(name="const", bufs=1) as cpool, \
             tc.tile_pool(name="msgs", bufs=1) as pool, \
             tc.tile_pool(name="epi", bufs=3) as epool, \
             tc.tile_pool(name="psum", bufs=2, space="PSUM") as ppool:
            wn_sb = cpool.tile([P, slot_tot], f16)
            W2b_sb = cpool.tile([P, EPB * C], f32)
            b1c_sb = cpool.tile([EPB * H, 1], f32)
            ident = cpool.tile([P, P], f32)
            out_all = cpool.tile([P, NWIN * C], f16)
            nc.sync.dma_start(out=wn_sb[:], in_=wn_d[:])
            nc.sync.dma_start(out=W2b_sb[:], in_=W2b_d[:])
            nc.sync.dma_start(out=b1c_sb[:], in_=b1c_d[:])
            make_identity(nc, ident[:])

            def body():
                # dedicated per-group tiles; all stream DMAs issued up-front
                # on the Sync HWDGE ring (they drain in order, smallest group
                # first) so compute never waits on a just-issued DMA
                msgs = {}
                for g in reversed(range(NGRP)):
                    nw, L, off = int(nwg[g]), int(Lg[g]), int(goff[g])
                    msg = pool.tile([P, nw * H * L], f16, tag=f"msg{g}")
                    nc.sync.dma_start(out=msg[:],
                                      in_=msg_d[:, off * H:(off + nw * L) * H])
                    msgs[g] = msg
                for g in reversed(range(NGRP)):
                    nw, L, off = int(nwg[g]), int(Lg[g]), int(goff[g])
                    msg = msgs[g]
                    m4 = msg[:].rearrange("p (w f s) -> p w f s",
                                          w=nw, f=H)
                    wb = (wn_sb[:, off:off + nw * L]
                          .rearrange("p (w s) -> p w s", w=nw)
                          .unsqueeze(2).to_broadcast([P, nw, H, L]))
                    nc.vector.tensor_tensor(out=m4, in0=m4, in1=wb, op=mult)
                    h = L // 2
                    nc.vector.tensor_tensor(out=m4[:, :, :, :h],
                                            in0=m4[:, :, :, :h],
                                            in1=m4[:, :, :, h:2 * h], op=add)
                    q = h // 2
                    nc.vector.tensor_tensor(out=m4[:, :, :, :q],
                                            in0=m4[:, :, :, :q],
                                            in1=m4[:, :, :, q:2 * q], op=add)
                    agg = epool.tile([P, GW * H], f32, tag="agg")
                    a3 = agg[:, :nw * H].rearrange("p (w f) -> p w f", w=nw)
                    nc.vector.tensor_reduce(out=a3, in_=m4[:, :, :, :q],
                                            axis=mybir.AxisListType.X, op=add)
                    # epilogue in EPB-window sub-blocks (EPB*H = 128):
                    # transpose agg -> zT = relu(aggT + 64*b1) fused on Act
                    # (bias is per-partition in the transposed layout);
                    # the 1/64 is folded into W2b
                    for j in range(0, nw, EPB):
                        nb = min(EPB, nw - j)
                        zT_p = ppool.tile([EPB * H, P], f32, tag="zT")
                        nc.tensor.transpose(
                            out=zT_p[:nb * H, :],
                            in_=agg[:, j * H:(j + nb) * H],
                            identity=ident[:])
                        zT = epool.tile([EPB * H, P], f32, tag="zTs")
                        nc.scalar.activation(
                            out=zT[:nb * H, :], in_=zT_p[:nb * H, :],
                            func=mybir.ActivationFunctionType.Relu,
                            bias=b1c_sb[:nb * H, :])
                        o8 = ppool.tile([P, EPB * C], f32, tag="o8")
                        nc.tensor.matmul(out=o8[:, :nb * C],
                                         lhsT=zT[:nb * H, :],
                                         rhs=W2b_sb[:nb * H, :nb * C],
                                         start=True, stop=True)
                        col = (GW * g + j) * C
                        nc.scalar.copy(out=out_all[:, col:col + nb * C],
                                       in_=o8[:, :nb * C])
                nc.scalar.dma_start(out=out_d[:], in_=out_all[:])

            if loop_reps == 1:
                body()
            else:
                with tc.For_i(0, loop_reps, 1, staggered_reset=True):
                    body()
    nc.compile()
    return nc


def build_stageC(slot_tot, Lg, nwg, goff, loop_reps=1):
    """Layer-2 weighted segment-sum over 2-wide fp16 messages, then
    log_softmax(agg/64 + b2) -> f32 [P, NWIN*C]."""
    nc = _BaccSharedExpLn("TRN2", target_bir_lowering=False, debug=False,
                          num_devices=NCORES)
    f32, f16 = mybir.dt.float32, mybir.dt.float16
    msg_d = nc.dram_tensor("msg", [P, slot_tot * C], f16, kind="ExternalInput").ap()
    wn_d = nc.dram_tensor("wn", [P, slot_tot], f16, kind="ExternalInput").ap()
    b2_d = nc.dram_tensor("b2", [P, C], f32, kind="ExternalInput").ap()
    out_d = nc.dram_tensor("out", [P, NWIN * C], f32, kind="ExternalOutput").ap()

    maxblk = max(int(nwg[g]) * int(Lg[g]) for g in range(NGRP)) * C
    mult, add = mybir.AluOpType.mult, mybir.AluOpType.add
    sub = mybir.AluOpType.subtract

    with tile.TileContext(nc) as tc:
        with tc.tile_pool(name="const", bufs=1) as cpool, \
             tc.tile_pool(name="msgs", bufs=1) as pool, \
             tc.tile_pool(name="epi", bufs=2) as epool:
            wn_sb = cpool.tile([P, slot_tot], f16)
            b2_sb = cpool.tile([P, C], f32)
            agg_all = cpool.tile([P, NWIN * C], f32)
            out_all = cpool.tile([P, NWIN * C], f32)
            nc.sync.dma_start(out=wn_sb[:], in_=wn_d[:])
            nc.sync.dma_start(out=b2_sb[:], in_=b2_d[:])

            def body():
                msgs = {}
                for g in reversed(range(NGRP)):
                    nw, L, off = int(nwg[g]), int(Lg[g]), int(goff[g])
                    msg = pool.tile([P, nw * C * L], f16, tag=f"msg{g}")
                    nc.sync.dma_start(out=msg[:],
                                      in_=msg_d[:, off * C:(off + nw * L) * C])
                    msgs[g] = msg
                for g in reversed(range(NGRP)):
                    nw, L, off = int(nwg[g]), int(Lg[g]), int(goff[g])
                    msg = msgs[g]
                    m4 = msg[:].rearrange("p (w f s) -> p w f s",
                                          w=nw, f=C)
                    wb = (wn_sb[:, off:off + nw * L]
                          .rearrange("p (w s) -> p w s", w=nw)
                          .unsqueeze(2).to_broadcast([P, nw, C, L]))
                    nc.vector.tensor_tensor(out=m4, in0=m4, in1=wb, op=mult)
                    h = L // 2
                    nc.vector.tensor_tensor(out=m4[:, :, :, :h],
                                            in0=m4[:, :, :, :h],
                                            in1=m4[:, :, :, h:2 * h], op=add)
                    q = h // 2
                    nc.vector.tensor_tensor(out=m4[:, :, :, :q],
                                            in0=m4[:, :, :, :q],
                                            in1=m4[:, :, :, q:2 * q], op=add)
                    a3 = agg_all[:, GW * g * C:(GW * g + nw) * C].rearrange(
                        "p (w c) -> p w c", w=nw)
                    nc.vector.tensor_reduce(out=a3, in_=m4[:, :, :, :q],
                                            axis=mybir.AxisListType.X, op=add)
                # batched epilogue (one Exp / one Ln per iteration: the Act
                # engine reloads its function table on every func switch,
                # 1.28 us each)
                t = epool.tile([P, NWIN * C], f32, tag="t")
                a3 = agg_all[:].rearrange("p (w c) -> p w c", c=C)
                t3 = t[:].rearrange("p (w c) -> p w c", c=C)
                b2b = b2_sb[:].unsqueeze(1).to_broadcast([P, NWIN, C])
                nc.vector.scalar_tensor_tensor(out=t3, in0=a3,
                                               scalar=1.0 / WSCALE,
                                               in1=b2b, op0=mult, op1=add)
                rmax = epool.tile([P, NWIN], f32, tag="rmax")
                nc.vector.tensor_reduce(out=rmax[:], in_=t3,
                                        axis=mybir.AxisListType.X,
                                        op=mybir.AluOpType.max)
                rb = rmax[:].unsqueeze(2).to_broadcast([P, NWIN, C])
                nc.vector.tensor_tensor(out=t3, in0=t3, in1=rb, op=sub)
                e = epool.tile([P, NWIN * C], f32, tag="e")
                nc.scalar.activation(out=e[:], in_=t[:],
                                     func=mybir.ActivationFunctionType.Exp)
                se = epool.tile([P, NWIN], f32, tag="se")
                nc.vector.tensor_reduce(
                    out=se[:], in_=e[:].rearrange("p (w c) -> p w c", c=C),
                    axis=mybir.AxisListType.X, op=add)
                lse = epool.tile([P, NWIN], f32, tag="lse")
                nc.scalar.activation(out=lse[:], in_=se[:],
                                     func=mybir.ActivationFunctionType.Ln)
                ob = out_all[:].rearrange("p (w c) -> p w c", c=C)
                lb = lse[:].unsqueeze(2).to_broadcast([P, NWIN, C])
                nc.vector.tensor_tensor(out=ob, in0=t3, in1=lb, op=sub)
                nc.scalar.dma_start(out=out_d[:], in_=out_all[:])

            if loop_reps == 1:
                body()
            else:
                with tc.For_i(0, loop_reps, 1, staggered_reset=True):
                    body()
    nc.compile()
    return nc


# ----------------------------------------------------------------------------
# Full model
# ----------------------------------------------------------------------------

_CACHE = {}


def get_programs(g, loop_reps=1):
    key = (g["slot_tot"], tuple(g["Lg"]), loop_reps)
    if key not in _CACHE:
        kA = build_stageA(loop_reps)
        kB = build_stageB(g["slot_tot"], g["Lg"], g["nwg"], g["goff"], loop_reps)
        kC = build_stageC(g["slot_tot"], g["Lg"], g["nwg"], g["goff"], loop_reps)
        _CACHE[key] = (kA, kB, kC)
    return _CACHE[key]


def kernel(x, edge_index, edge_weight, W1, b1, W2, b2):
    x = np.asarray(x, np.float32)
    W1 = np.asarray(W1, np.float32)
    b1 = np.asarray(b1, np.float32)
    W2 = np.asarray(W2, np.float32)
    b2 = np.asarray(b2, np.float32)

    g = preprocess_graph(edge_index, edge_weight)
    kA, kB, kC = get_programs(g, 1)
    cores = list(range(NCORES))

    # ---- stage A: H0 = X @ W1 (node-partitioned output) ----
    W1h = W1.astype(np.float16)
    inA = []
    for c in range(NCORES):
        xT = np.zeros((F_IN, DPC_PAD), np.float16)
        xT[:, :DPC] = x[c * DPC:(c + 1) * DPC].T.astype(np.float16)
        inA.append({"xT": xT, "W1": W1h})
    rA = bass_utils.run_bass_kernel_spmd(kA, inA, core_ids=cores)
    h0_full = np.empty((N, H), np.float16)
    for c in range(NCORES):
        arr = rA.results[c]["h0"].reshape(P, NWIN, H).transpose(1, 0, 2)
        h0_full[c * DPC:(c + 1) * DPC] = arr.reshape(DPC_PAD, H)[:DPC]

    # ---- host halo exchange 1 + stage B ----
    W2b = (np.kron(np.eye(EPB, dtype=np.float32), W2) / WSCALE).astype(np.float32)
    b1c = np.tile(b1 * WSCALE, EPB).astype(np.float32)[:, None]
    inB = []
    for c in range(NCORES):
        msg = gather_group_msgs(h0_full, g["sp"][c], g["Lg"], g["nwg"],
                                g["goff"], H)
        inB.append({"msg": msg, "wn": g["wn16"][c], "W2b": W2b, "b1c": b1c})
    rB = bass_utils.run_bass_kernel_spmd(kB, inB, core_ids=cores)
    z2_full = np.empty((N, C), np.float16)
    for c in range(NCORES):
        order, _ = g["perms"][c]
        z2_full[c * DPC:(c + 1) * DPC] = unpermute_out(
            rB.results[c]["out"], order, np.float16)

    # ---- host halo exchange 2 + stage C ----
    b2r = np.broadcast_to(b2, (P, C)).astype(np.float32).copy()
    inC = []
    for c in range(NCORES):
        msg = gather_group_msgs(z2_full, g["sp"][c], g["Lg"], g["nwg"],
                                g["goff"], C)
        inC.append({"msg": msg, "wn": g["wn16"][c], "b2": b2r})
    rC = bass_utils.run_bass_kernel_spmd(kC, inC, core_ids=cores)

    out = np.empty((N, C), np.float32)
    for c in range(NCORES):
        order, _ = g["perms"][c]
        out[c * DPC:(c + 1) * DPC] = unpermute_out(
            rC.results[c]["out"], order, np.float32)
    return out
